# revision 7
# baseline (speedup 1.0000x reference)
"""Trainium2 Bass kernel for nn_Decoder (AIR-style decoder: per-object MLP
decode -> spatial transformer paste -> depth-softmax compositing).

Sharding: 8 cores = (batch b = core//2) x (half of the 32 object slots).
Host gathers the *present* objects of each half (padded to a common even
count Pmax), ships scalar affine-marshalled grid params, and sums the two
half-canvases per batch at the end. All O(big) math runs on device:
  - what-decoder MLP (bf16 matmuls for the big w2 layer, fp32 accum)
  - sigmoid / exp / softmax normalization
  - separable bilinear STN as two matmul hops (fp32r):
      W[x,h]       = sum_y P[y,(c,x)] * AT[y,h]
      canvasT[w,h] = sum_{n,x} B[x,w] * W[x,h]
  - interpolation grids AT/B built on device: one DVE affine op over an
    iota table + ACT Abs + ACT Relu (hat = relu(1-|py - idx|)), with the
    per-object softmax weight folded into B via ACT scale/bias columns.
The channel loop interleaves the w2 stream with the STN of the previous
channel so DMA and PE overlap. Output per core: transposed partial canvas
[3, 416(w), 416(h)]; host sums core pairs and un-transposes.
"""

import os
import numpy as np
import ml_dtypes

import concourse.bacc as bacc
import concourse.tile as tile
import concourse.mybir as mybir
from concourse.bass_utils import run_bass_kernel_spmd

dt = mybir.dt
F32 = dt.float32
F32R = dt.float32r
BF16 = dt.bfloat16
AF = mybir.ActivationFunctionType
ALU = mybir.AluOpType

B, N, D, H1 = 4, 32, 64, 512
S = 64           # decoded patch size
IMG = 416        # canvas size
JD = 3 * S * S   # 12288 decoder output dim
CORES = 8
HALF = N // 2    # 16 object slots per core
EMPTY_DEPTH = -1000.0

_CACHE = {}


def _build(pmax):
    pairs = pmax // 2
    nc = bacc.Bacc("TRN2", target_bir_lowering=False, debug=False,
                   num_devices=CORES)

    # ---- per-core inputs ----
    zT_d = nc.declare_dram_parameter("zT", [D, pmax], F32R, isOutput=False)
    dall_d = nc.declare_dram_parameter("dall", [1, N], F32, isOutput=False)
    dsel_d = nc.declare_dram_parameter("dsel", [1, pmax], F32, isOutput=False)
    afA_d = nc.declare_dram_parameter("afA", [128, 2 * pairs], F32, isOutput=False)
    afB_d = nc.declare_dram_parameter("afB", [128, 2 * pairs], F32, isOutput=False)
    # ---- shared (same content on every core) ----
    w1_d = nc.declare_dram_parameter("w1", [D, H1], F32R, isOutput=False)
    b1c_d = nc.declare_dram_parameter("b1c", [128, 4], F32, isOutput=False)
    w2_d = nc.declare_dram_parameter("w2t", [4, 12, 128, 1024], BF16,
                                     isOutput=False)
    b2r_d = nc.declare_dram_parameter("b2r", [1, JD], F32R, isOutput=False)
    iota_d = nc.declare_dram_parameter("iota416", [128, IMG], F32, isOutput=False)
    inds_d = nc.declare_dram_parameter("inds", [1, 256], F32R, isOutput=False)
    onesP_d = nc.declare_dram_parameter("onesP", [1, pmax], F32R, isOutput=False)
    zer_d = nc.declare_dram_parameter("zeros", [64, 384 * 8], F32R,
                                      isOutput=False)
    # ---- output: transposed partial canvas ----
    out_d = nc.declare_dram_parameter("canvasT", [3, IMG, IMG], F32, isOutput=True)

    WCH = [(0, 128), (128, 128), (256, 128), (384, 32)]  # (w0, wsize) chunks

    with tile.TileContext(nc) as tc:
        with tc.tile_pool(name="const", bufs=1) as cp, \
             tc.tile_pool(name="w2s", bufs=2) as w2p, \
             tc.tile_pool(name="work", bufs=1) as wk, \
             tc.tile_pool(name="wpool", bufs=2) as wpl, \
             tc.tile_pool(name="evac", bufs=2) as ev, \
             tc.tile_pool(name="psA", bufs=2, space="PSUM") as psA, \
             tc.tile_pool(name="psW", bufs=2, space="PSUM") as psW, \
             tc.tile_pool(name="psC", bufs=3, space="PSUM") as psC:

            # ---------- load constants / inputs ----------
            zT_t = cp.tile([D, pmax], F32R, tag="zT")
            nc.sync.dma_start(zT_t[:], zT_d[:])
            w1_t = cp.tile([D, H1], F32R, tag="w1")
            nc.sync.dma_start(w1_t[:], w1_d[:])
            b1c_t = cp.tile([128, 4], F32, tag="b1c")
            nc.sync.dma_start(b1c_t[:], b1c_d[:])
            iota_t = cp.tile([128, IMG], F32, tag="iota")
            nc.sync.dma_start(iota_t[:], iota_d[:])
            inds_t = cp.tile([1, 256], F32R, tag="inds")
            nc.sync.dma_start(inds_t[:], inds_d[:])
            onesP_t = cp.tile([1, pmax], F32R, tag="onesP")
            nc.sync.dma_start(onesP_t[:], onesP_d[:])
            dall_t = cp.tile([1, N], F32, tag="dall")
            nc.sync.dma_start(dall_t[:], dall_d[:])
            dsel_t = cp.tile([1, pmax], F32, tag="dsel")
            nc.sync.dma_start(dsel_t[:], dsel_d[:])
            afA_t = cp.tile([128, 2 * pairs], F32, tag="afA")
            nc.sync.dma_start(afA_t[:], afA_d[:])
            afB_t = cp.tile([128, 2 * pairs], F32, tag="afB")
            nc.sync.dma_start(afB_t[:], afB_d[:])

            # ---------- MLP hid: hidT = relu(w1.T @ z + b1), [H1, pmax] ----
            hidT_t = cp.tile([128, 4 * pmax], BF16, tag="hidT")
            for kc in range(4):
                hp = psA.tile([128, pmax], F32, tag="psA")
                nc.tensor.matmul(hp[:], w1_t[:, kc * 128:(kc + 1) * 128],
                                 zT_t[:], start=True, stop=True)
                nc.scalar.activation(hidT_t[:, kc * pmax:(kc + 1) * pmax], hp[:],
                                     AF.Relu, bias=b1c_t[:, kc:kc + 1])

            # ---------- softmax weights s (selected objects) ----------
            eall_t = wk.tile([1, N], F32, tag="eall")
            nc.scalar.activation(eall_t[:], dall_t[:], AF.Exp)
            z_t = wk.tile([1, 1], F32, tag="z")
            nc.vector.tensor_reduce(z_t[:], eall_t[:], mybir.AxisListType.X,
                                    ALU.add)
            zg_t = wk.tile([1, 1], F32, tag="zg")
            nc.vector.tensor_scalar_add(zg_t[:], z_t[:], 1e-30)
            zr_t = wk.tile([1, 1], F32, tag="zr")
            nc.vector.reciprocal(zr_t[:], zg_t[:])
            esel_t = wk.tile([1, pmax], F32, tag="esel")
            nc.scalar.activation(esel_t[:], dsel_t[:], AF.Exp)
            srow_t = wk.tile([1, pmax], F32R, tag="srow")
            nc.vector.tensor_scalar_mul(srow_t[:], esel_t[:], zr_t[:, 0:1])
            # broadcast to per-partition columns: scol[p', pair] (lo=even obj)
            sp = psA.tile([128, pairs], F32, tag="psA")
            nc.tensor.matmul(sp[:], inds_t[0:1, 0:128],
                             srow_t[0:1, 0:pmax:2], start=True, stop=False)
            nc.tensor.matmul(sp[:], inds_t[0:1, 128:256],
                             srow_t[0:1, 1:pmax:2], start=False, stop=True)
            scol_t = wk.tile([128, pairs], F32, tag="scol")
            nc.scalar.copy(scol_t[:], sp[:])
            nscol_t = wk.tile([128, pairs], F32, tag="nscol")
            nc.scalar.mul(nscol_t[:], scol_t[:], -1.0)

            # ---------- grids ----------
            AT_t = wk.tile([128, IMG * pairs], F32R, tag="AT")
            Bg_t = wk.tile([128, IMG * pairs], F32R, tag="Bg")
            for p in range(pairs):
                sl = slice(p * IMG, (p + 1) * IMG)
                mA = ev.tile([128, IMG], F32, tag="m")
                nc.vector.tensor_scalar(mA[:], iota_t[:],
                                        afA_t[:, 2 * p:2 * p + 1],
                                        afA_t[:, 2 * p + 1:2 * p + 2],
                                        ALU.mult, ALU.add)
                tA = ev.tile([128, IMG], F32, tag="t")
                nc.scalar.activation(tA[:], mA[:], AF.Abs)
                nc.scalar.activation(AT_t[:, sl], tA[:], AF.Relu,
                                     bias=1.0, scale=-1.0)
                mB = ev.tile([128, IMG], F32, tag="m")
                nc.vector.tensor_scalar(mB[:], iota_t[:],
                                        afB_t[:, 2 * p:2 * p + 1],
                                        afB_t[:, 2 * p + 1:2 * p + 2],
                                        ALU.mult, ALU.add)
                tB = ev.tile([128, IMG], F32, tag="t")
                nc.scalar.activation(tB[:], mB[:], AF.Abs)
                nc.scalar.activation(Bg_t[:, sl], tB[:], AF.Relu,
                                     bias=scol_t[:, p:p + 1],
                                     scale=nscol_t[:, p:p + 1])
            # stage odd-object A-grid rows (partitions 64:128) to base 0
            AT2_t = wk.tile([64, IMG * pairs], F32R, tag="AT2")
            for p in range(pairs):
                psl = slice(p * IMG, (p + 1) * IMG)
                nc.sync.dma_start(AT2_t[:, psl], AT_t[64:128, psl])

            # ---------- per-channel: dec -> patches -> step1 -> step2 ------
            # Ppe/Ppo [64(y), 384*pairs]: even/odd object of each pair, both
            # at base partition 0 (mixed-base psum accumulation is fatal on
            # HW). cols c*128+[0:64] = even object's x, +[64:128] = odd's x;
            # the other half of each 128-col block stays zero (zero-padded
            # lhsT trick so the step-1 mm pair writes disjoint output rows).
            Ppe_t = wk.tile([64, 384 * pairs], F32R, tag="Ppe")
            Ppo_t = wk.tile([64, 384 * pairs], F32R, tag="Ppo")
            nc.sync.dma_start(Ppe_t[:], zer_d[:, 0:384 * pairs])
            nc.sync.dma_start(Ppo_t[:], zer_d[:, 0:384 * pairs])

            for c in range(3):
                # --- dec_c = sigmoid(hid @ w2[:, c-block] + b2) ---
                dec_c = w2p.tile([pmax, 4096], F32R, tag="dec_c")
                for gg in range(4):
                    g = c * 4 + gg
                    w2g = []
                    for kt in range(4):
                        t = w2p.tile([128, 1024], BF16, tag=f"w2_{kt}")
                        nc.sync.dma_start(t[:], w2_d[kt, g])
                        w2g.append(t)
                    b2g = w2p.tile([1, 1024], F32R, tag="b2g")
                    nc.sync.dma_start(b2g[:],
                                      b2r_d[0:1, g * 1024:(g + 1) * 1024])
                    for jl in range(2):
                        dp = psA.tile([pmax, 512], F32, tag="psA")
                        for kt in range(4):
                            nc.tensor.matmul(
                                dp[:], hidT_t[:, kt * pmax:(kt + 1) * pmax],
                                w2g[kt][:, jl * 512:(jl + 1) * 512],
                                start=(kt == 0), stop=False)
                        nc.tensor.matmul(dp[:], onesP_t[:],
                                         b2g[:, jl * 512:(jl + 1) * 512],
                                         start=False, stop=True)
                        nc.scalar.activation(
                            dec_c[:, (gg * 2 + jl) * 512:
                                  (gg * 2 + jl + 1) * 512],
                            dp[:], AF.Sigmoid)
                # --- patch gather for this channel ---
                for p in range(pairs):
                    base = p * 384
                    for hf, Pt in ((0, Ppe_t), (1, Ppo_t)):
                        n = 2 * p + hf
                        src = dec_c[n:n + 1, :].rearrange(
                            "o (y x) -> o y x", y=S, x=S)
                        dst = Pt[:, base + c * 128 + hf * 64:
                                 base + c * 128 + (hf + 1) * 64]
                        nc.sync.dma_start(dst.opt(), src.opt())
                # --- step 1: W[pair] [128(x-halves), IMG(h)] ---
                Wt = wpl.tile([128, IMG * pairs], F32R, tag="W")
                for p in range(pairs):
                    base = p * 384
                    psl = slice(p * IMG, (p + 1) * IMG)
                    wp = psW.tile([128, IMG], F32, tag="psW")
                    nc.tensor.matmul(
                        wp[:], Ppe_t[:, base + c * 128:base + (c + 1) * 128],
                        AT_t[0:64, psl], start=True, stop=False)
                    nc.tensor.matmul(
                        wp[:], Ppo_t[:, base + c * 128:base + (c + 1) * 128],
                        AT2_t[:, psl], start=False, stop=True)
                    nc.vector.tensor_copy(Wt[:, psl], wp[:])
                # --- step 2: canvasT[c][w, h] ---
                for (w0, wsz) in WCH:
                    cv = psC.tile([wsz, IMG], F32, tag="psC")
                    for p in range(pairs):
                        nc.tensor.matmul(
                            cv[:], Bg_t[:, p * IMG + w0:p * IMG + w0 + wsz],
                            Wt[:, p * IMG:(p + 1) * IMG],
                            start=(p == 0), stop=(p == pairs - 1))
                    ot = ev.tile([wsz, IMG], F32, tag="cv")
                    nc.vector.tensor_copy(ot[:], cv[:])
                    nc.sync.dma_start(out_d[c, w0:w0 + wsz, :], ot[:])

    nc.compile()
    return nc


def kernel(z_where, z_present, z_what, z_depth, w1, b1, w2, b2):
    z_where = np.asarray(z_where, dtype=np.float32)
    z_present = np.asarray(z_present)
    z_what = np.asarray(z_what, dtype=np.float32)
    z_depth = np.asarray(z_depth, dtype=np.float32)
    w1 = np.ascontiguousarray(np.asarray(w1, dtype=np.float32))
    b1 = np.asarray(b1, dtype=np.float32)
    w2 = np.ascontiguousarray(np.asarray(w2, dtype=np.float32))
    b2 = np.asarray(b2, dtype=np.float32)

    pres = z_present.reshape(B, N) > 0
    depth = z_depth.reshape(B, N)

    # per-core object selection (present only, sorted by y-center)
    sels = []
    for k in range(CORES):
        b, half = k // 2, k % 2
        cand = [n for n in range(half * HALF, (half + 1) * HALF) if pres[b, n]]
        cand.sort(key=lambda n: z_where[b, n, 1])
        sels.append(cand)
    pmax = max(2, max((len(s) for s in sels), default=0))
    if pmax % 2:
        pmax += 1
    pairs = pmax // 2

    # shared constants
    b1c = np.ascontiguousarray(b1.reshape(4, 128).T)
    b2r = b2.reshape(1, JD)
    iota416 = np.ascontiguousarray(
        np.broadcast_to(np.arange(IMG, dtype=np.float32), (128, IMG)))
    inds = np.zeros((1, 256), np.float32)
    inds[0, 0:64] = 1.0
    inds[0, 192:256] = 1.0
    onesP = np.ones((1, pmax), np.float32)
    zeros64 = np.zeros((64, 384 * 8), np.float32)
    w2t = np.ascontiguousarray(
        w2.reshape(4, 128, 12, 1024).transpose(0, 2, 1, 3)).astype(
            ml_dtypes.bfloat16)
    pidx = np.arange(128, dtype=np.float32) % 64

    in_maps = []
    for k in range(CORES):
        b = k // 2
        sel = sels[k]
        P = len(sel)
        zT = np.zeros((D, pmax), np.float32)
        if P:
            zT[:, :P] = z_what[b, sel].T
        dall = np.where(pres[b], depth[b], EMPTY_DEPTH).astype(
            np.float32).reshape(1, N)
        dsel = np.full((1, pmax), EMPTY_DEPTH, np.float32)
        dsel[0, :P] = depth[b, sel]

        afA = np.zeros((128, 2 * pairs), np.float32)
        afB = np.zeros((128, 2 * pairs), np.float32)
        for p in range(pairs):
            for hf in range(2):
                i = 2 * p + hf
                rows = slice(hf * 64, (hf + 1) * 64)
                if i < P:
                    xc, yc, ww, hh = z_where[b, sel[i]]
                    sx = max(ww, 1e-3)
                    sy = max(hh, 1e-3)
                    tx = 2.0 * xc - 1.0
                    ty = 2.0 * yc - 1.0
                    aY = 63.0 / (415.0 * sy)
                    bY = 31.5 * ((-1.0 - ty) / sy + 1.0)
                    aX = 63.0 / (415.0 * sx)
                    bX = 31.5 * ((-1.0 - tx) / sx + 1.0)
                else:
                    aY = aX = 0.0
                    bY = bX = -5.0
                afA[rows, 2 * p] = aY
                afA[rows, 2 * p + 1] = bY - pidx[rows]
                afB[rows, 2 * p] = aX
                afB[rows, 2 * p + 1] = bX - pidx[rows]

        in_maps.append({
            "zT": zT, "dall": dall, "dsel": dsel, "afA": afA, "afB": afB,
            "w1": w1, "b1c": b1c, "w2t": w2t, "b2r": b2r,
            "iota416": iota416, "inds": inds, "onesP": onesP,
            "zeros": zeros64,
        })

    if pmax not in _CACHE:
        _CACHE[pmax] = _build(pmax)
    nc = _CACHE[pmax]

    trace = os.environ.get("BASS_KERNEL_TRACE", "0") == "1"
    res = run_bass_kernel_spmd(nc, in_maps, list(range(CORES)), trace=trace)
    if trace and res.exec_time_ns is not None:
        print(f"HW exec time: {res.exec_time_ns} ns")

    recon = np.zeros((B, 3, IMG, IMG), np.float32)
    for b in range(B):
        ct = res.results[2 * b]["canvasT"] + res.results[2 * b + 1]["canvasT"]
        recon[b] = ct.transpose(0, 2, 1)
    return recon


# revision 8
# speedup vs baseline: 1.1180x; 1.1180x over previous
"""Trainium2 Bass kernel for nn_Decoder (AIR-style decoder: per-object MLP
decode -> spatial transformer paste -> depth-softmax compositing).

Sharding: 8 cores = (batch b = core//2) x (half of the 32 object slots).
Host gathers the *present* objects of each half (padded to a common even
count Pmax), ships scalar affine-marshalled grid params, and sums the two
half-canvases per batch at the end. All O(big) math runs on device:
  - what-decoder MLP (bf16 matmuls for the big w2 layer, fp32 accum)
  - sigmoid / exp / softmax normalization
  - separable bilinear STN as two matmul hops (fp32r):
      W[x,h]       = sum_y P[y,(c,x)] * AT[y,h]
      canvasT[w,h] = sum_{n,x} B[x,w] * W[x,h]
  - interpolation grids AT/B built on device: one DVE affine op over an
    iota table + ACT Abs + ACT Relu (hat = relu(1-|py - idx|)), with the
    per-object softmax weight folded into B via ACT scale/bias columns.
The channel loop interleaves the w2 stream with the STN of the previous
channel so DMA and PE overlap. Output per core: transposed partial canvas
[3, 416(w), 416(h)]; host sums core pairs and un-transposes.
"""

import os
import numpy as np
import ml_dtypes

import concourse.bacc as bacc
import concourse.tile as tile
import concourse.mybir as mybir
from concourse.bass_utils import run_bass_kernel_spmd

dt = mybir.dt
F32 = dt.float32
F32R = dt.float32r
BF16 = dt.bfloat16
AF = mybir.ActivationFunctionType
ALU = mybir.AluOpType

B, N, D, H1 = 4, 32, 64, 512
S = 64           # decoded patch size
IMG = 416        # canvas size
JD = 3 * S * S   # 12288 decoder output dim
CORES = 8
HALF = N // 2    # 16 object slots per core
EMPTY_DEPTH = -1000.0

_CACHE = {}


def _build(pmax, hasb2):
    pairs = pmax // 2
    nc = bacc.Bacc("TRN2", target_bir_lowering=False, debug=False,
                   num_devices=CORES)

    # ---- per-core inputs ----
    zT_d = nc.declare_dram_parameter("zT", [D, pmax], F32R, isOutput=False)
    dall_d = nc.declare_dram_parameter("dall", [1, N], F32, isOutput=False)
    dsel_d = nc.declare_dram_parameter("dsel", [1, pmax], F32, isOutput=False)
    afA_d = nc.declare_dram_parameter("afA", [128, 2 * pairs], F32, isOutput=False)
    afB_d = nc.declare_dram_parameter("afB", [128, 2 * pairs], F32, isOutput=False)
    # ---- shared (same content on every core) ----
    w1_d = nc.declare_dram_parameter("w1", [D, H1], F32R, isOutput=False)
    b1c_d = nc.declare_dram_parameter("b1c", [128, 4], F32, isOutput=False)
    w2_d = nc.declare_dram_parameter("w2t", [4, 12, 128, 1024], BF16,
                                     isOutput=False)
    b2r_d = nc.declare_dram_parameter("b2r", [1, JD], F32R, isOutput=False)
    iota_d = nc.declare_dram_parameter("iota416", [128, IMG], F32, isOutput=False)
    inds_d = nc.declare_dram_parameter("inds", [1, 256], F32R, isOutput=False)
    onesP_d = nc.declare_dram_parameter("onesP", [1, pmax], F32R, isOutput=False)
    zer_d = nc.declare_dram_parameter("zeros", [64, 384 * 8], BF16,
                                      isOutput=False)
    # ---- output: transposed partial canvas ----
    out_d = nc.declare_dram_parameter("canvasT", [3, IMG, IMG], F32, isOutput=True)

    WCH = [(0, 128), (128, 128), (256, 128), (384, 32)]  # (w0, wsize) chunks

    with tile.TileContext(nc) as tc:
        with tc.tile_pool(name="const", bufs=1) as cp, \
             tc.tile_pool(name="w2s", bufs=2) as w2p, \
             tc.tile_pool(name="work", bufs=1) as wk, \
             tc.tile_pool(name="wpool", bufs=2) as wpl, \
             tc.tile_pool(name="evac", bufs=2) as ev, \
             tc.tile_pool(name="psA", bufs=2, space="PSUM") as psA, \
             tc.tile_pool(name="psW", bufs=2, space="PSUM") as psW, \
             tc.tile_pool(name="psC", bufs=3, space="PSUM") as psC:

            # ---------- load constants / inputs ----------
            zT_t = cp.tile([D, pmax], F32R, tag="zT")
            nc.sync.dma_start(zT_t[:], zT_d[:])
            w1_t = cp.tile([D, H1], F32R, tag="w1")
            nc.sync.dma_start(w1_t[:], w1_d[:])
            b1c_t = cp.tile([128, 4], F32, tag="b1c")
            nc.sync.dma_start(b1c_t[:], b1c_d[:])
            iota_t = cp.tile([128, IMG], F32, tag="iota")
            nc.sync.dma_start(iota_t[:], iota_d[:])
            inds_t = cp.tile([1, 256], F32R, tag="inds")
            nc.sync.dma_start(inds_t[:], inds_d[:])
            onesP_t = cp.tile([1, pmax], F32R, tag="onesP")
            nc.sync.dma_start(onesP_t[:], onesP_d[:])
            dall_t = cp.tile([1, N], F32, tag="dall")
            nc.sync.dma_start(dall_t[:], dall_d[:])
            dsel_t = cp.tile([1, pmax], F32, tag="dsel")
            nc.sync.dma_start(dsel_t[:], dsel_d[:])
            afA_t = cp.tile([128, 2 * pairs], F32, tag="afA")
            nc.sync.dma_start(afA_t[:], afA_d[:])
            afB_t = cp.tile([128, 2 * pairs], F32, tag="afB")
            nc.sync.dma_start(afB_t[:], afB_d[:])

            # ---------- MLP hid: hidT = relu(w1.T @ z + b1), [H1, pmax] ----
            hidT_t = cp.tile([128, 4 * pmax], BF16, tag="hidT")
            for kc in range(4):
                hp = psA.tile([128, pmax], F32, tag="psA")
                nc.tensor.matmul(hp[:], w1_t[:, kc * 128:(kc + 1) * 128],
                                 zT_t[:], start=True, stop=True)
                nc.scalar.activation(hidT_t[:, kc * pmax:(kc + 1) * pmax], hp[:],
                                     AF.Relu, bias=b1c_t[:, kc:kc + 1])

            # ---------- softmax weights s (selected objects) ----------
            eall_t = wk.tile([1, N], F32, tag="eall")
            nc.scalar.activation(eall_t[:], dall_t[:], AF.Exp)
            z_t = wk.tile([1, 1], F32, tag="z")
            nc.vector.tensor_reduce(z_t[:], eall_t[:], mybir.AxisListType.X,
                                    ALU.add)
            zg_t = wk.tile([1, 1], F32, tag="zg")
            nc.vector.tensor_scalar_add(zg_t[:], z_t[:], 1e-30)
            zr_t = wk.tile([1, 1], F32, tag="zr")
            nc.vector.reciprocal(zr_t[:], zg_t[:])
            esel_t = wk.tile([1, pmax], F32, tag="esel")
            nc.scalar.activation(esel_t[:], dsel_t[:], AF.Exp)
            srow_t = wk.tile([1, pmax], F32R, tag="srow")
            nc.vector.tensor_scalar_mul(srow_t[:], esel_t[:], zr_t[:, 0:1])
            # broadcast to per-partition columns: scol[p', pair] (lo=even obj)
            sp = psA.tile([128, pairs], F32, tag="psA")
            nc.tensor.matmul(sp[:], inds_t[0:1, 0:128],
                             srow_t[0:1, 0:pmax:2], start=True, stop=False)
            nc.tensor.matmul(sp[:], inds_t[0:1, 128:256],
                             srow_t[0:1, 1:pmax:2], start=False, stop=True)
            scol_t = wk.tile([128, pairs], F32, tag="scol")
            nc.scalar.copy(scol_t[:], sp[:])
            nscol_t = wk.tile([128, pairs], F32, tag="nscol")
            nc.scalar.mul(nscol_t[:], scol_t[:], -1.0)

            # ---------- grids ----------
            AT_t = wk.tile([128, IMG * pairs], BF16, tag="AT")
            Bg_t = wk.tile([128, IMG * pairs], BF16, tag="Bg")
            for p in range(pairs):
                sl = slice(p * IMG, (p + 1) * IMG)
                mA = ev.tile([128, IMG], F32, tag="m")
                nc.vector.tensor_scalar(mA[:], iota_t[:],
                                        afA_t[:, 2 * p:2 * p + 1],
                                        afA_t[:, 2 * p + 1:2 * p + 2],
                                        ALU.mult, ALU.add)
                tA = ev.tile([128, IMG], F32, tag="t")
                nc.scalar.activation(tA[:], mA[:], AF.Abs)
                nc.scalar.activation(AT_t[:, sl], tA[:], AF.Relu,
                                     bias=1.0, scale=-1.0)
                mB = ev.tile([128, IMG], F32, tag="m")
                nc.vector.tensor_scalar(mB[:], iota_t[:],
                                        afB_t[:, 2 * p:2 * p + 1],
                                        afB_t[:, 2 * p + 1:2 * p + 2],
                                        ALU.mult, ALU.add)
                tB = ev.tile([128, IMG], F32, tag="t")
                nc.scalar.activation(tB[:], mB[:], AF.Abs)
                nc.scalar.activation(Bg_t[:, sl], tB[:], AF.Relu,
                                     bias=scol_t[:, p:p + 1],
                                     scale=nscol_t[:, p:p + 1])
            # stage odd-object A-grid rows (partitions 64:128) to base 0
            AT2_t = wk.tile([64, IMG * pairs], BF16, tag="AT2")
            for p in range(pairs):
                psl = slice(p * IMG, (p + 1) * IMG)
                nc.sync.dma_start(AT2_t[:, psl], AT_t[64:128, psl])

            # ---------- per-channel: dec -> patches -> step1 -> step2 ------
            # Ppe/Ppo [64(y), 384*pairs]: even/odd object of each pair, both
            # at base partition 0 (mixed-base psum accumulation is fatal on
            # HW). cols c*128+[0:64] = even object's x, +[64:128] = odd's x;
            # the other half of each 128-col block stays zero (zero-padded
            # lhsT trick so the step-1 mm pair writes disjoint output rows).
            Ppe_t = wk.tile([64, 384 * pairs], BF16, tag="Ppe")
            Ppo_t = wk.tile([64, 384 * pairs], BF16, tag="Ppo")
            nc.sync.dma_start(Ppe_t[:], zer_d[:, 0:384 * pairs])
            nc.sync.dma_start(Ppo_t[:], zer_d[:, 0:384 * pairs])

            for c in range(3):
                # --- dec_c = sigmoid(hid @ w2[:, c-block] + b2) ---
                dec_c = w2p.tile([pmax, 4096], BF16, tag="dec_c")
                for gg in range(4):
                    g = c * 4 + gg
                    w2g = []
                    for kt in range(4):
                        t = w2p.tile([128, 1024], BF16, tag=f"w2_{kt}")
                        nc.sync.dma_start(t[:], w2_d[kt, g])
                        w2g.append(t)
                    if hasb2:
                        b2g = w2p.tile([1, 1024], F32R, tag="b2g")
                        nc.sync.dma_start(b2g[:],
                                          b2r_d[0:1, g * 1024:(g + 1) * 1024])
                    for jl in range(2):
                        dp = psA.tile([pmax, 512], F32, tag="psA")
                        for kt in range(4):
                            nc.tensor.matmul(
                                dp[:], hidT_t[:, kt * pmax:(kt + 1) * pmax],
                                w2g[kt][:, jl * 512:(jl + 1) * 512],
                                start=(kt == 0), stop=(kt == 3 and not hasb2))
                        if hasb2:
                            nc.tensor.matmul(dp[:], onesP_t[:],
                                             b2g[:, jl * 512:(jl + 1) * 512],
                                             start=False, stop=True)
                        nc.scalar.activation(
                            dec_c[:, (gg * 2 + jl) * 512:
                                  (gg * 2 + jl + 1) * 512],
                            dp[:], AF.Sigmoid)
                # --- patch gather for this channel ---
                for p in range(pairs):
                    base = p * 384
                    for hf, Pt in ((0, Ppe_t), (1, Ppo_t)):
                        n = 2 * p + hf
                        src = dec_c[n:n + 1, :].rearrange(
                            "o (y x) -> o y x", y=S, x=S)
                        dst = Pt[:, base + c * 128 + hf * 64:
                                 base + c * 128 + (hf + 1) * 64]
                        nc.sync.dma_start(dst.opt(), src.opt())
                # --- step 1: W[pair] [128(x-halves), IMG(h)] ---
                Wt = wpl.tile([128, IMG * pairs], BF16, tag="W")
                for p in range(pairs):
                    base = p * 384
                    psl = slice(p * IMG, (p + 1) * IMG)
                    wp = psW.tile([128, IMG], F32, tag="psW")
                    nc.tensor.matmul(
                        wp[:], Ppe_t[:, base + c * 128:base + (c + 1) * 128],
                        AT_t[0:64, psl], start=True, stop=False)
                    nc.tensor.matmul(
                        wp[:], Ppo_t[:, base + c * 128:base + (c + 1) * 128],
                        AT2_t[:, psl], start=False, stop=True)
                    nc.vector.tensor_copy(Wt[:, psl], wp[:])
                # --- step 2: canvasT[c][w, h] ---
                for (w0, wsz) in WCH:
                    cv = psC.tile([wsz, IMG], F32, tag="psC")
                    for p in range(pairs):
                        nc.tensor.matmul(
                            cv[:], Bg_t[:, p * IMG + w0:p * IMG + w0 + wsz],
                            Wt[:, p * IMG:(p + 1) * IMG],
                            start=(p == 0), stop=(p == pairs - 1))
                    ot = ev.tile([wsz, IMG], F32, tag="cv")
                    nc.vector.tensor_copy(ot[:], cv[:])
                    nc.sync.dma_start(out_d[c, w0:w0 + wsz, :], ot[:])

    nc.compile()
    return nc


def kernel(z_where, z_present, z_what, z_depth, w1, b1, w2, b2):
    z_where = np.asarray(z_where, dtype=np.float32)
    z_present = np.asarray(z_present)
    z_what = np.asarray(z_what, dtype=np.float32)
    z_depth = np.asarray(z_depth, dtype=np.float32)
    w1 = np.ascontiguousarray(np.asarray(w1, dtype=np.float32))
    b1 = np.asarray(b1, dtype=np.float32)
    w2 = np.ascontiguousarray(np.asarray(w2, dtype=np.float32))
    b2 = np.asarray(b2, dtype=np.float32)

    pres = z_present.reshape(B, N) > 0
    depth = z_depth.reshape(B, N)

    # per-core object selection (present only, sorted by y-center)
    sels = []
    for k in range(CORES):
        b, half = k // 2, k % 2
        cand = [n for n in range(half * HALF, (half + 1) * HALF) if pres[b, n]]
        cand.sort(key=lambda n: z_where[b, n, 1])
        sels.append(cand)
    pmax = max(2, max((len(s) for s in sels), default=0))
    if pmax % 2:
        pmax += 1
    pairs = pmax // 2

    # shared constants
    b1c = np.ascontiguousarray(b1.reshape(4, 128).T)
    b2r = b2.reshape(1, JD)
    iota416 = np.ascontiguousarray(
        np.broadcast_to(np.arange(IMG, dtype=np.float32), (128, IMG)))
    inds = np.zeros((1, 256), np.float32)
    inds[0, 0:64] = 1.0
    inds[0, 192:256] = 1.0
    onesP = np.ones((1, pmax), np.float32)
    zeros64 = np.zeros((64, 384 * 8), ml_dtypes.bfloat16)
    w2t = np.ascontiguousarray(
        w2.reshape(4, 128, 12, 1024).transpose(0, 2, 1, 3)).astype(
            ml_dtypes.bfloat16)
    pidx = np.arange(128, dtype=np.float32) % 64

    in_maps = []
    for k in range(CORES):
        b = k // 2
        sel = sels[k]
        P = len(sel)
        zT = np.zeros((D, pmax), np.float32)
        if P:
            zT[:, :P] = z_what[b, sel].T
        dall = np.where(pres[b], depth[b], EMPTY_DEPTH).astype(
            np.float32).reshape(1, N)
        dsel = np.full((1, pmax), EMPTY_DEPTH, np.float32)
        dsel[0, :P] = depth[b, sel]

        afA = np.zeros((128, 2 * pairs), np.float32)
        afB = np.zeros((128, 2 * pairs), np.float32)
        for p in range(pairs):
            for hf in range(2):
                i = 2 * p + hf
                rows = slice(hf * 64, (hf + 1) * 64)
                if i < P:
                    xc, yc, ww, hh = z_where[b, sel[i]]
                    sx = max(ww, 1e-3)
                    sy = max(hh, 1e-3)
                    tx = 2.0 * xc - 1.0
                    ty = 2.0 * yc - 1.0
                    aY = 63.0 / (415.0 * sy)
                    bY = 31.5 * ((-1.0 - ty) / sy + 1.0)
                    aX = 63.0 / (415.0 * sx)
                    bX = 31.5 * ((-1.0 - tx) / sx + 1.0)
                else:
                    aY = aX = 0.0
                    bY = bX = -5.0
                afA[rows, 2 * p] = aY
                afA[rows, 2 * p + 1] = bY - pidx[rows]
                afB[rows, 2 * p] = aX
                afB[rows, 2 * p + 1] = bX - pidx[rows]

        in_maps.append({
            "zT": zT, "dall": dall, "dsel": dsel, "afA": afA, "afB": afB,
            "w1": w1, "b1c": b1c, "w2t": w2t, "b2r": b2r,
            "iota416": iota416, "inds": inds, "onesP": onesP,
            "zeros": zeros64,
        })

    hasb2 = bool(np.any(b2))
    key = (pmax, hasb2)
    if key not in _CACHE:
        _CACHE[key] = _build(pmax, hasb2)
    nc = _CACHE[key]

    trace = os.environ.get("BASS_KERNEL_TRACE", "0") == "1"
    res = run_bass_kernel_spmd(nc, in_maps, list(range(CORES)), trace=trace)
    if trace and res.exec_time_ns is not None:
        print(f"HW exec time: {res.exec_time_ns} ns")

    recon = np.zeros((B, 3, IMG, IMG), np.float32)
    for b in range(B):
        ct = res.results[2 * b]["canvasT"] + res.results[2 * b + 1]["canvasT"]
        recon[b] = ct.transpose(0, 2, 1)
    return recon


# revision 9
# speedup vs baseline: 1.1694x; 1.0460x over previous
"""Trainium2 Bass kernel for nn_Decoder (AIR-style decoder: per-object MLP
decode -> spatial transformer paste -> depth-softmax compositing).

Sharding: 8 cores = (batch b = core//2) x (half of the 32 object slots).
Host gathers the *present* objects of each half (padded to a common even
count Pmax), ships scalar affine-marshalled grid params, and sums the two
half-canvases per batch at the end. All O(big) math runs on device:
  - what-decoder MLP (bf16 matmuls for the big w2 layer, fp32 accum)
  - sigmoid / exp / softmax normalization
  - separable bilinear STN as two matmul hops (fp32r):
      W[x,h]       = sum_y P[y,(c,x)] * AT[y,h]
      canvasT[w,h] = sum_{n,x} B[x,w] * W[x,h]
  - interpolation grids AT/B built on device: one DVE affine op over an
    iota table + ACT Abs + ACT Relu (hat = relu(1-|py - idx|)), with the
    per-object softmax weight folded into B via ACT scale/bias columns.
The channel loop interleaves the w2 stream with the STN of the previous
channel so DMA and PE overlap. Output per core: transposed partial canvas
[3, 416(w), 416(h)]; host sums core pairs and un-transposes.
"""

import os
import numpy as np

import concourse.bacc as bacc
import concourse.tile as tile
import concourse.mybir as mybir
from concourse.bass_utils import run_bass_kernel_spmd

dt = mybir.dt
F32 = dt.float32
F32R = dt.float32r
BF16 = dt.float16  # fp16: same PE rate as bf16, 8 more mantissa bits
AF = mybir.ActivationFunctionType
ALU = mybir.AluOpType

B, N, D, H1 = 4, 32, 64, 512
S = 64           # decoded patch size
IMG = 416        # canvas size
JD = 3 * S * S   # 12288 decoder output dim
CORES = 8
HALF = N // 2    # 16 object slots per core
EMPTY_DEPTH = -1000.0

_CACHE = {}


def _build(pmax, hasb2):
    pairs = pmax // 2
    nc = bacc.Bacc("TRN2", target_bir_lowering=False, debug=False,
                   num_devices=CORES)

    # ---- per-core inputs ----
    zT_d = nc.declare_dram_parameter("zT", [D, pmax], F32R, isOutput=False)
    dall_d = nc.declare_dram_parameter("dall", [1, N], F32, isOutput=False)
    dsel_d = nc.declare_dram_parameter("dsel", [1, pmax], F32, isOutput=False)
    afA_d = nc.declare_dram_parameter("afA", [128, 2 * pairs], F32, isOutput=False)
    afB_d = nc.declare_dram_parameter("afB", [128, 2 * pairs], F32, isOutput=False)
    # ---- shared (same content on every core) ----
    w1_d = nc.declare_dram_parameter("w1", [D, H1], F32R, isOutput=False)
    b1c_d = nc.declare_dram_parameter("b1c", [128, 4], F32, isOutput=False)
    w2_d = nc.declare_dram_parameter("w2t", [4, 12, 128, 1024], BF16,
                                     isOutput=False)
    b2r_d = nc.declare_dram_parameter("b2r", [1, JD], F32R, isOutput=False)
    iota_d = nc.declare_dram_parameter("iota416", [128, IMG], F32, isOutput=False)
    inds_d = nc.declare_dram_parameter("inds", [1, 256], F32R, isOutput=False)
    onesP_d = nc.declare_dram_parameter("onesP", [1, pmax], F32R, isOutput=False)
    zer_d = nc.declare_dram_parameter("zeros", [64, 384 * 8], BF16,
                                      isOutput=False)
    # ---- output: transposed partial canvas ----
    out_d = nc.declare_dram_parameter("canvasT", [3, IMG, IMG], F32, isOutput=True)

    WCH = [(0, 128), (128, 128), (256, 128), (384, 32)]  # (w0, wsize) chunks

    with tile.TileContext(nc) as tc:
        with tc.tile_pool(name="const", bufs=1) as cp, \
             tc.tile_pool(name="w2s", bufs=2) as w2p, \
             tc.tile_pool(name="work", bufs=1) as wk, \
             tc.tile_pool(name="wpool", bufs=2) as wpl, \
             tc.tile_pool(name="evac", bufs=2) as ev, \
             tc.tile_pool(name="psA", bufs=2, space="PSUM") as psA, \
             tc.tile_pool(name="psW", bufs=2, space="PSUM") as psW, \
             tc.tile_pool(name="psC", bufs=3, space="PSUM") as psC:

            # ---------- load constants / inputs ----------
            zT_t = cp.tile([D, pmax], F32R, tag="zT")
            nc.sync.dma_start(zT_t[:], zT_d[:])
            w1_t = cp.tile([D, H1], F32R, tag="w1")
            nc.sync.dma_start(w1_t[:], w1_d[:])
            b1c_t = cp.tile([128, 4], F32, tag="b1c")
            nc.sync.dma_start(b1c_t[:], b1c_d[:])
            iota_t = cp.tile([128, IMG], F32, tag="iota")
            nc.sync.dma_start(iota_t[:], iota_d[:])
            inds_t = cp.tile([1, 256], F32R, tag="inds")
            nc.sync.dma_start(inds_t[:], inds_d[:])
            onesP_t = cp.tile([1, pmax], F32R, tag="onesP")
            nc.sync.dma_start(onesP_t[:], onesP_d[:])
            dall_t = cp.tile([1, N], F32, tag="dall")
            nc.sync.dma_start(dall_t[:], dall_d[:])
            dsel_t = cp.tile([1, pmax], F32, tag="dsel")
            nc.sync.dma_start(dsel_t[:], dsel_d[:])
            afA_t = cp.tile([128, 2 * pairs], F32, tag="afA")
            nc.sync.dma_start(afA_t[:], afA_d[:])
            afB_t = cp.tile([128, 2 * pairs], F32, tag="afB")
            nc.sync.dma_start(afB_t[:], afB_d[:])

            # ---------- MLP hid: hidT = relu(w1.T @ z + b1), [H1, pmax] ----
            hidT_t = cp.tile([128, 4 * pmax], BF16, tag="hidT")
            for kc in range(4):
                hp = psA.tile([128, pmax], F32, tag="psA")
                nc.tensor.matmul(hp[:], w1_t[:, kc * 128:(kc + 1) * 128],
                                 zT_t[:], start=True, stop=True)
                nc.scalar.activation(hidT_t[:, kc * pmax:(kc + 1) * pmax], hp[:],
                                     AF.Relu, bias=b1c_t[:, kc:kc + 1])

            # ---------- softmax weights s (selected objects) ----------
            eall_t = wk.tile([1, N], F32, tag="eall")
            nc.scalar.activation(eall_t[:], dall_t[:], AF.Exp)
            z_t = wk.tile([1, 1], F32, tag="z")
            nc.vector.tensor_reduce(z_t[:], eall_t[:], mybir.AxisListType.X,
                                    ALU.add)
            zg_t = wk.tile([1, 1], F32, tag="zg")
            nc.vector.tensor_scalar_add(zg_t[:], z_t[:], 1e-30)
            zr_t = wk.tile([1, 1], F32, tag="zr")
            nc.vector.reciprocal(zr_t[:], zg_t[:])
            esel_t = wk.tile([1, pmax], F32, tag="esel")
            nc.scalar.activation(esel_t[:], dsel_t[:], AF.Exp)
            srow_t = wk.tile([1, pmax], F32R, tag="srow")
            nc.vector.tensor_scalar_mul(srow_t[:], esel_t[:], zr_t[:, 0:1])
            # broadcast to per-partition columns: scol[p', pair] (lo=even obj)
            sp = psA.tile([128, pairs], F32, tag="psA")
            nc.tensor.matmul(sp[:], inds_t[0:1, 0:128],
                             srow_t[0:1, 0:pmax:2], start=True, stop=False)
            nc.tensor.matmul(sp[:], inds_t[0:1, 128:256],
                             srow_t[0:1, 1:pmax:2], start=False, stop=True)
            scol_t = wk.tile([128, pairs], F32, tag="scol")
            nc.scalar.copy(scol_t[:], sp[:])
            nscol_t = wk.tile([128, pairs], F32, tag="nscol")
            nc.scalar.mul(nscol_t[:], scol_t[:], -1.0)

            # ---------- grids ----------
            AT_t = wk.tile([128, IMG * pairs], BF16, tag="AT")
            Bg_t = wk.tile([128, IMG * pairs], BF16, tag="Bg")
            for p in range(pairs):
                sl = slice(p * IMG, (p + 1) * IMG)
                mA = ev.tile([128, IMG], F32, tag="m")
                nc.vector.tensor_scalar(mA[:], iota_t[:],
                                        afA_t[:, 2 * p:2 * p + 1],
                                        afA_t[:, 2 * p + 1:2 * p + 2],
                                        ALU.mult, ALU.add)
                tA = ev.tile([128, IMG], F32, tag="t")
                nc.scalar.activation(tA[:], mA[:], AF.Abs)
                nc.scalar.activation(AT_t[:, sl], tA[:], AF.Relu,
                                     bias=1.0, scale=-1.0)
                mB = ev.tile([128, IMG], F32, tag="m")
                nc.vector.tensor_scalar(mB[:], iota_t[:],
                                        afB_t[:, 2 * p:2 * p + 1],
                                        afB_t[:, 2 * p + 1:2 * p + 2],
                                        ALU.mult, ALU.add)
                tB = ev.tile([128, IMG], F32, tag="t")
                nc.scalar.activation(tB[:], mB[:], AF.Abs)
                nc.scalar.activation(Bg_t[:, sl], tB[:], AF.Relu,
                                     bias=scol_t[:, p:p + 1],
                                     scale=nscol_t[:, p:p + 1])
            # stage odd-object A-grid rows (partitions 64:128) to base 0
            AT2_t = wk.tile([64, IMG * pairs], BF16, tag="AT2")
            for p in range(pairs):
                psl = slice(p * IMG, (p + 1) * IMG)
                nc.sync.dma_start(AT2_t[:, psl], AT_t[64:128, psl])

            # ---------- per-channel: dec -> patches -> step1 -> step2 ------
            # Ppe/Ppo [64(y), 384*pairs]: even/odd object of each pair, both
            # at base partition 0 (mixed-base psum accumulation is fatal on
            # HW). cols c*128+[0:64] = even object's x, +[64:128] = odd's x;
            # the other half of each 128-col block stays zero (zero-padded
            # lhsT trick so the step-1 mm pair writes disjoint output rows).
            Ppe_t = wk.tile([64, 384 * pairs], BF16, tag="Ppe")
            Ppo_t = wk.tile([64, 384 * pairs], BF16, tag="Ppo")
            nc.sync.dma_start(Ppe_t[:], zer_d[:, 0:384 * pairs])
            nc.sync.dma_start(Ppo_t[:], zer_d[:, 0:384 * pairs])

            for c in range(3):
                # --- dec_c = sigmoid(hid @ w2[:, c-block] + b2) ---
                dec_c = w2p.tile([pmax, 4096], BF16, tag="dec_c")
                for gg in range(4):
                    g = c * 4 + gg
                    w2g = []
                    for kt in range(4):
                        t = w2p.tile([128, 1024], BF16, tag=f"w2_{kt}")
                        nc.sync.dma_start(t[:], w2_d[kt, g])
                        w2g.append(t)
                    if hasb2:
                        b2g = w2p.tile([1, 1024], F32R, tag="b2g")
                        nc.sync.dma_start(b2g[:],
                                          b2r_d[0:1, g * 1024:(g + 1) * 1024])
                    for jl in range(2):
                        dp = psA.tile([pmax, 512], F32, tag="psA")
                        for kt in range(4):
                            nc.tensor.matmul(
                                dp[:], hidT_t[:, kt * pmax:(kt + 1) * pmax],
                                w2g[kt][:, jl * 512:(jl + 1) * 512],
                                start=(kt == 0), stop=(kt == 3 and not hasb2))
                        if hasb2:
                            nc.tensor.matmul(dp[:], onesP_t[:],
                                             b2g[:, jl * 512:(jl + 1) * 512],
                                             start=False, stop=True)
                        nc.scalar.activation(
                            dec_c[:, (gg * 2 + jl) * 512:
                                  (gg * 2 + jl + 1) * 512],
                            dp[:], AF.Sigmoid)
                # --- patch gather for this channel ---
                for p in range(pairs):
                    base = p * 384
                    for hf, Pt in ((0, Ppe_t), (1, Ppo_t)):
                        n = 2 * p + hf
                        src = dec_c[n:n + 1, :].rearrange(
                            "o (y x) -> o y x", y=S, x=S)
                        dst = Pt[:, base + c * 128 + hf * 64:
                                 base + c * 128 + (hf + 1) * 64]
                        nc.sync.dma_start(dst.opt(), src.opt())
                # --- step 1: W[pair] [128(x-halves), IMG(h)] ---
                Wt = wpl.tile([128, IMG * pairs], BF16, tag="W")
                for p in range(pairs):
                    base = p * 384
                    psl = slice(p * IMG, (p + 1) * IMG)
                    wp = psW.tile([128, IMG], F32, tag="psW")
                    nc.tensor.matmul(
                        wp[:], Ppe_t[:, base + c * 128:base + (c + 1) * 128],
                        AT_t[0:64, psl], start=True, stop=False)
                    nc.tensor.matmul(
                        wp[:], Ppo_t[:, base + c * 128:base + (c + 1) * 128],
                        AT2_t[:, psl], start=False, stop=True)
                    nc.vector.tensor_copy(Wt[:, psl], wp[:])
                # --- step 2: canvasT[c][w, h] ---
                for (w0, wsz) in WCH:
                    cv = psC.tile([wsz, IMG], F32, tag="psC")
                    for p in range(pairs):
                        nc.tensor.matmul(
                            cv[:], Bg_t[:, p * IMG + w0:p * IMG + w0 + wsz],
                            Wt[:, p * IMG:(p + 1) * IMG],
                            start=(p == 0), stop=(p == pairs - 1))
                    ot = ev.tile([wsz, IMG], F32, tag="cv")
                    nc.vector.tensor_copy(ot[:], cv[:])
                    nc.sync.dma_start(out_d[c, w0:w0 + wsz, :], ot[:])

    nc.compile()
    return nc


def kernel(z_where, z_present, z_what, z_depth, w1, b1, w2, b2):
    z_where = np.asarray(z_where, dtype=np.float32)
    z_present = np.asarray(z_present)
    z_what = np.asarray(z_what, dtype=np.float32)
    z_depth = np.asarray(z_depth, dtype=np.float32)
    w1 = np.ascontiguousarray(np.asarray(w1, dtype=np.float32))
    b1 = np.asarray(b1, dtype=np.float32)
    w2 = np.ascontiguousarray(np.asarray(w2, dtype=np.float32))
    b2 = np.asarray(b2, dtype=np.float32)

    pres = z_present.reshape(B, N) > 0
    depth = z_depth.reshape(B, N)

    # per-core object selection (present only, sorted by y-center)
    sels = []
    for k in range(CORES):
        b, half = k // 2, k % 2
        cand = [n for n in range(half * HALF, (half + 1) * HALF) if pres[b, n]]
        cand.sort(key=lambda n: z_where[b, n, 1])
        sels.append(cand)
    pmax = max(2, max((len(s) for s in sels), default=0))
    if pmax % 2:
        pmax += 1
    pairs = pmax // 2

    # shared constants
    b1c = np.ascontiguousarray(b1.reshape(4, 128).T)
    b2r = b2.reshape(1, JD)
    iota416 = np.ascontiguousarray(
        np.broadcast_to(np.arange(IMG, dtype=np.float32), (128, IMG)))
    inds = np.zeros((1, 256), np.float32)
    inds[0, 0:64] = 1.0
    inds[0, 192:256] = 1.0
    onesP = np.ones((1, pmax), np.float32)
    zeros64 = np.zeros((64, 384 * 8), np.float16)
    w2t = np.ascontiguousarray(
        w2.reshape(4, 128, 12, 1024).transpose(0, 2, 1, 3)).astype(
            np.float16)
    pidx = np.arange(128, dtype=np.float32) % 64

    in_maps = []
    for k in range(CORES):
        b = k // 2
        sel = sels[k]
        P = len(sel)
        zT = np.zeros((D, pmax), np.float32)
        if P:
            zT[:, :P] = z_what[b, sel].T
        dall = np.where(pres[b], depth[b], EMPTY_DEPTH).astype(
            np.float32).reshape(1, N)
        dsel = np.full((1, pmax), EMPTY_DEPTH, np.float32)
        dsel[0, :P] = depth[b, sel]

        afA = np.zeros((128, 2 * pairs), np.float32)
        afB = np.zeros((128, 2 * pairs), np.float32)
        for p in range(pairs):
            for hf in range(2):
                i = 2 * p + hf
                rows = slice(hf * 64, (hf + 1) * 64)
                if i < P:
                    xc, yc, ww, hh = z_where[b, sel[i]]
                    sx = max(ww, 1e-3)
                    sy = max(hh, 1e-3)
                    tx = 2.0 * xc - 1.0
                    ty = 2.0 * yc - 1.0
                    aY = 63.0 / (415.0 * sy)
                    bY = 31.5 * ((-1.0 - ty) / sy + 1.0)
                    aX = 63.0 / (415.0 * sx)
                    bX = 31.5 * ((-1.0 - tx) / sx + 1.0)
                else:
                    aY = aX = 0.0
                    bY = bX = -5.0
                afA[rows, 2 * p] = aY
                afA[rows, 2 * p + 1] = bY - pidx[rows]
                afB[rows, 2 * p] = aX
                afB[rows, 2 * p + 1] = bX - pidx[rows]

        in_maps.append({
            "zT": zT, "dall": dall, "dsel": dsel, "afA": afA, "afB": afB,
            "w1": w1, "b1c": b1c, "w2t": w2t, "b2r": b2r,
            "iota416": iota416, "inds": inds, "onesP": onesP,
            "zeros": zeros64,
        })

    hasb2 = bool(np.any(b2))
    key = (pmax, hasb2)
    if key not in _CACHE:
        _CACHE[key] = _build(pmax, hasb2)
    nc = _CACHE[key]

    trace = os.environ.get("BASS_KERNEL_TRACE", "0") == "1"
    res = run_bass_kernel_spmd(nc, in_maps, list(range(CORES)), trace=trace)
    if trace and res.exec_time_ns is not None:
        print(f"HW exec time: {res.exec_time_ns} ns")

    recon = np.zeros((B, 3, IMG, IMG), np.float32)
    for b in range(B):
        ct = res.results[2 * b]["canvasT"] + res.results[2 * b + 1]["canvasT"]
        recon[b] = ct.transpose(0, 2, 1)
    return recon


# revision 10
# speedup vs baseline: 1.2756x; 1.0908x over previous
"""Trainium2 Bass kernel for nn_Decoder (AIR-style decoder: per-object MLP
decode -> spatial transformer paste -> depth-softmax compositing).

Sharding: 8 cores = (batch b = core//2) x (half of the 32 object slots).
Host gathers the *present* objects of each half (padded to a common even
count Pmax), ships scalar affine-marshalled grid params, and sums the two
half-canvases per batch at the end. All O(big) math runs on device:
  - what-decoder MLP (bf16 matmuls for the big w2 layer, fp32 accum)
  - sigmoid / exp / softmax normalization
  - separable bilinear STN as two matmul hops (fp32r):
      W[x,h]       = sum_y P[y,(c,x)] * AT[y,h]
      canvasT[w,h] = sum_{n,x} B[x,w] * W[x,h]
  - interpolation grids AT/B built on device: one DVE affine op over an
    iota table + ACT Abs + ACT Relu (hat = relu(1-|py - idx|)), with the
    per-object softmax weight folded into B via ACT scale/bias columns.
The channel loop interleaves the w2 stream with the STN of the previous
channel so DMA and PE overlap. Output per core: transposed partial canvas
[3, 416(w), 416(h)]; host sums core pairs and un-transposes.
"""

import os
import numpy as np

import concourse.bacc as bacc
import concourse.tile as tile
import concourse.mybir as mybir
from concourse.bass_utils import run_bass_kernel_spmd

dt = mybir.dt
F32 = dt.float32
F32R = dt.float32r
BF16 = dt.float16  # fp16: same PE rate as bf16, 8 more mantissa bits
AF = mybir.ActivationFunctionType
ALU = mybir.AluOpType

B, N, D, H1 = 4, 32, 64, 512
S = 64           # decoded patch size
IMG = 416        # canvas size
JD = 3 * S * S   # 12288 decoder output dim
CORES = 8
HALF = N // 2    # 16 object slots per core
EMPTY_DEPTH = -1000.0

_CACHE = {}


def _build(pmax, hasb2):
    pairs = pmax // 2
    nc = bacc.Bacc("TRN2", target_bir_lowering=False, debug=False,
                   num_devices=CORES)

    # ---- per-core inputs ----
    zT_d = nc.declare_dram_parameter("zT", [D, pmax], F32R, isOutput=False)
    dall_d = nc.declare_dram_parameter("dall", [1, N], F32, isOutput=False)
    dsel_d = nc.declare_dram_parameter("dsel", [1, pmax], F32, isOutput=False)
    afA_d = nc.declare_dram_parameter("afA", [128, 2 * pairs], F32, isOutput=False)
    afB_d = nc.declare_dram_parameter("afB", [128, 2 * pairs], F32, isOutput=False)
    # ---- shared (same content on every core) ----
    w1_d = nc.declare_dram_parameter("w1", [D, H1], F32R, isOutput=False)
    b1c_d = nc.declare_dram_parameter("b1c", [128, 4], F32, isOutput=False)
    w2_d = nc.declare_dram_parameter("w2t", [4, 12, 128, 1024], BF16,
                                     isOutput=False)
    b2r_d = nc.declare_dram_parameter("b2r", [1, JD], F32R, isOutput=False)
    iota_d = nc.declare_dram_parameter("iota416", [128, IMG], F32, isOutput=False)
    inds_d = nc.declare_dram_parameter("inds", [1, 256], F32R, isOutput=False)
    onesP_d = nc.declare_dram_parameter("onesP", [1, pmax], F32R, isOutput=False)
    zer_d = nc.declare_dram_parameter("zeros", [64, 384 * 8], BF16,
                                      isOutput=False)
    # ---- output: transposed partial canvas ----
    out_d = nc.declare_dram_parameter("canvasT", [3, IMG, IMG], F32, isOutput=True)

    WCH = [(0, 128), (128, 128), (256, 128), (384, 32)]  # (w0, wsize) chunks

    with tile.TileContext(nc) as tc:
        with tc.tile_pool(name="const", bufs=1) as cp, \
             tc.tile_pool(name="w2s", bufs=3) as w2p, \
             tc.tile_pool(name="work", bufs=1) as wk, \
             tc.tile_pool(name="wpool", bufs=2) as wpl, \
             tc.tile_pool(name="evac", bufs=2) as ev, \
             tc.tile_pool(name="psA", bufs=3, space="PSUM") as psA, \
             tc.tile_pool(name="psW", bufs=2, space="PSUM") as psW, \
             tc.tile_pool(name="psC", bufs=2, space="PSUM") as psC:

            # ---------- load constants / inputs ----------
            zT_t = cp.tile([D, pmax], F32R, tag="zT")
            nc.gpsimd.dma_start(zT_t[:], zT_d[:])
            w1_t = cp.tile([D, H1], F32R, tag="w1")
            nc.gpsimd.dma_start(w1_t[:], w1_d[:])
            b1c_t = cp.tile([128, 4], F32, tag="b1c")
            nc.gpsimd.dma_start(b1c_t[:], b1c_d[:])
            iota_t = cp.tile([128, IMG], F32, tag="iota")
            nc.gpsimd.dma_start(iota_t[:], iota_d[:])
            inds_t = cp.tile([1, 256], F32R, tag="inds")
            nc.gpsimd.dma_start(inds_t[:], inds_d[:])
            onesP_t = cp.tile([1, pmax], F32R, tag="onesP")
            nc.gpsimd.dma_start(onesP_t[:], onesP_d[:])
            dall_t = cp.tile([1, N], F32, tag="dall")
            nc.gpsimd.dma_start(dall_t[:], dall_d[:])
            dsel_t = cp.tile([1, pmax], F32, tag="dsel")
            nc.gpsimd.dma_start(dsel_t[:], dsel_d[:])
            afA_t = cp.tile([128, 2 * pairs], F32, tag="afA")
            nc.gpsimd.dma_start(afA_t[:], afA_d[:])
            afB_t = cp.tile([128, 2 * pairs], F32, tag="afB")
            nc.gpsimd.dma_start(afB_t[:], afB_d[:])

            # ---------- MLP hid: hidT = relu(w1.T @ z + b1), [H1, pmax] ----
            hidT_t = cp.tile([128, 4 * pmax], BF16, tag="hidT")
            for kc in range(4):
                hp = psA.tile([128, pmax], F32, tag="psA")
                nc.tensor.matmul(hp[:], w1_t[:, kc * 128:(kc + 1) * 128],
                                 zT_t[:], start=True, stop=True)
                nc.scalar.activation(hidT_t[:, kc * pmax:(kc + 1) * pmax], hp[:],
                                     AF.Relu, bias=b1c_t[:, kc:kc + 1])

            # ---------- softmax weights s (selected objects) ----------
            eall_t = wk.tile([1, N], F32, tag="eall")
            nc.scalar.activation(eall_t[:], dall_t[:], AF.Exp)
            z_t = wk.tile([1, 1], F32, tag="z")
            nc.vector.tensor_reduce(z_t[:], eall_t[:], mybir.AxisListType.X,
                                    ALU.add)
            zg_t = wk.tile([1, 1], F32, tag="zg")
            nc.vector.tensor_scalar_add(zg_t[:], z_t[:], 1e-30)
            zr_t = wk.tile([1, 1], F32, tag="zr")
            nc.vector.reciprocal(zr_t[:], zg_t[:])
            esel_t = wk.tile([1, pmax], F32, tag="esel")
            nc.scalar.activation(esel_t[:], dsel_t[:], AF.Exp)
            srow_t = wk.tile([1, pmax], F32R, tag="srow")
            nc.vector.tensor_scalar_mul(srow_t[:], esel_t[:], zr_t[:, 0:1])
            # broadcast to per-partition columns: scol[p', pair] (lo=even obj)
            sp = psA.tile([128, pairs], F32, tag="psA")
            nc.tensor.matmul(sp[:], inds_t[0:1, 0:128],
                             srow_t[0:1, 0:pmax:2], start=True, stop=False)
            nc.tensor.matmul(sp[:], inds_t[0:1, 128:256],
                             srow_t[0:1, 1:pmax:2], start=False, stop=True)
            scol_t = wk.tile([128, pairs], F32, tag="scol")
            nc.scalar.copy(scol_t[:], sp[:])
            nscol_t = wk.tile([128, pairs], F32, tag="nscol")
            nc.scalar.mul(nscol_t[:], scol_t[:], -1.0)

            # ---------- grids ----------
            AT_t = wk.tile([128, IMG * pairs], BF16, tag="AT")
            Bg_t = wk.tile([128, IMG * pairs], BF16, tag="Bg")
            for p in range(pairs):
                sl = slice(p * IMG, (p + 1) * IMG)
                mA = ev.tile([128, IMG], F32, tag="m")
                nc.vector.tensor_scalar(mA[:], iota_t[:],
                                        afA_t[:, 2 * p:2 * p + 1],
                                        afA_t[:, 2 * p + 1:2 * p + 2],
                                        ALU.mult, ALU.add)
                tA = ev.tile([128, IMG], F32, tag="t")
                nc.scalar.activation(tA[:], mA[:], AF.Abs)
                nc.scalar.activation(AT_t[:, sl], tA[:], AF.Relu,
                                     bias=1.0, scale=-1.0)
                mB = ev.tile([128, IMG], F32, tag="m")
                nc.vector.tensor_scalar(mB[:], iota_t[:],
                                        afB_t[:, 2 * p:2 * p + 1],
                                        afB_t[:, 2 * p + 1:2 * p + 2],
                                        ALU.mult, ALU.add)
                tB = ev.tile([128, IMG], F32, tag="t")
                nc.scalar.activation(tB[:], mB[:], AF.Abs)
                nc.scalar.activation(Bg_t[:, sl], tB[:], AF.Relu,
                                     bias=scol_t[:, p:p + 1],
                                     scale=nscol_t[:, p:p + 1])
            # stage odd-object A-grid rows (partitions 64:128) to base 0
            AT2_t = wk.tile([64, IMG * pairs], BF16, tag="AT2")
            for p in range(pairs):
                psl = slice(p * IMG, (p + 1) * IMG)
                nc.gpsimd.dma_start(AT2_t[:, psl], AT_t[64:128, psl])

            # ---------- per-channel: dec -> patches -> step1 -> step2 ------
            # Ppe/Ppo [64(y), 384*pairs]: even/odd object of each pair, both
            # at base partition 0 (mixed-base psum accumulation is fatal on
            # HW). cols c*128+[0:64] = even object's x, +[64:128] = odd's x;
            # the other half of each 128-col block stays zero (zero-padded
            # lhsT trick so the step-1 mm pair writes disjoint output rows).
            Ppe_t = wk.tile([64, 384 * pairs], BF16, tag="Ppe")
            Ppo_t = wk.tile([64, 384 * pairs], BF16, tag="Ppo")
            nc.gpsimd.dma_start(Ppe_t[:], zer_d[:, 0:384 * pairs])
            nc.gpsimd.dma_start(Ppo_t[:], zer_d[:, 0:384 * pairs])

            for c in range(3):
                # --- dec_c = sigmoid(hid @ w2[:, c-block] + b2) ---
                dec_c = w2p.tile([pmax, 4096], BF16, tag="dec_c")
                for gg in range(4):
                    g = c * 4 + gg
                    w2g = []
                    for kt in range(4):
                        t = w2p.tile([128, 1024], BF16, tag=f"w2_{kt}")
                        nc.sync.dma_start(t[:], w2_d[kt, g])
                        w2g.append(t)
                    if hasb2:
                        b2g = w2p.tile([1, 1024], F32R, tag="b2g")
                        nc.sync.dma_start(b2g[:],
                                          b2r_d[0:1, g * 1024:(g + 1) * 1024])
                    for jl in range(2):
                        dp = psA.tile([pmax, 512], F32, tag="psA")
                        for kt in range(4):
                            nc.tensor.matmul(
                                dp[:], hidT_t[:, kt * pmax:(kt + 1) * pmax],
                                w2g[kt][:, jl * 512:(jl + 1) * 512],
                                start=(kt == 0), stop=(kt == 3 and not hasb2))
                        if hasb2:
                            nc.tensor.matmul(dp[:], onesP_t[:],
                                             b2g[:, jl * 512:(jl + 1) * 512],
                                             start=False, stop=True)
                        nc.scalar.activation(
                            dec_c[:, (gg * 2 + jl) * 512:
                                  (gg * 2 + jl + 1) * 512],
                            dp[:], AF.Sigmoid)
                # --- patch gather for this channel ---
                for p in range(pairs):
                    base = p * 384
                    for hf, Pt in ((0, Ppe_t), (1, Ppo_t)):
                        n = 2 * p + hf
                        src = dec_c[n:n + 1, :].rearrange(
                            "o (y x) -> o y x", y=S, x=S)
                        dst = Pt[:, base + c * 128 + hf * 64:
                                 base + c * 128 + (hf + 1) * 64]
                        nc.gpsimd.dma_start(dst.opt(), src.opt())
                # --- step 1: W[pair] [128(x-halves), IMG(h)] ---
                Wt = wpl.tile([128, IMG * pairs], BF16, tag="W")
                for p in range(pairs):
                    base = p * 384
                    psl = slice(p * IMG, (p + 1) * IMG)
                    wp = psW.tile([128, IMG], F32, tag="psW")
                    nc.tensor.matmul(
                        wp[:], Ppe_t[:, base + c * 128:base + (c + 1) * 128],
                        AT_t[0:64, psl], start=True, stop=False)
                    nc.tensor.matmul(
                        wp[:], Ppo_t[:, base + c * 128:base + (c + 1) * 128],
                        AT2_t[:, psl], start=False, stop=True)
                    nc.vector.tensor_copy(Wt[:, psl], wp[:])
                # --- step 2: canvasT[c][w, h] ---
                for (w0, wsz) in WCH:
                    cv = psC.tile([wsz, IMG], F32, tag="psC")
                    for p in range(pairs):
                        nc.tensor.matmul(
                            cv[:], Bg_t[:, p * IMG + w0:p * IMG + w0 + wsz],
                            Wt[:, p * IMG:(p + 1) * IMG],
                            start=(p == 0), stop=(p == pairs - 1))
                    ot = ev.tile([wsz, IMG], F32, tag="cv")
                    nc.vector.tensor_copy(ot[:], cv[:])
                    nc.sync.dma_start(out_d[c, w0:w0 + wsz, :], ot[:])

    nc.compile()
    return nc


def kernel(z_where, z_present, z_what, z_depth, w1, b1, w2, b2):
    z_where = np.asarray(z_where, dtype=np.float32)
    z_present = np.asarray(z_present)
    z_what = np.asarray(z_what, dtype=np.float32)
    z_depth = np.asarray(z_depth, dtype=np.float32)
    w1 = np.ascontiguousarray(np.asarray(w1, dtype=np.float32))
    b1 = np.asarray(b1, dtype=np.float32)
    w2 = np.ascontiguousarray(np.asarray(w2, dtype=np.float32))
    b2 = np.asarray(b2, dtype=np.float32)

    pres = z_present.reshape(B, N) > 0
    depth = z_depth.reshape(B, N)

    # per-core object selection (present only, sorted by y-center)
    sels = []
    for k in range(CORES):
        b, half = k // 2, k % 2
        cand = [n for n in range(half * HALF, (half + 1) * HALF) if pres[b, n]]
        cand.sort(key=lambda n: z_where[b, n, 1])
        sels.append(cand)
    pmax = max(2, max((len(s) for s in sels), default=0))
    if pmax % 2:
        pmax += 1
    pairs = pmax // 2

    # shared constants
    b1c = np.ascontiguousarray(b1.reshape(4, 128).T)
    b2r = b2.reshape(1, JD)
    iota416 = np.ascontiguousarray(
        np.broadcast_to(np.arange(IMG, dtype=np.float32), (128, IMG)))
    inds = np.zeros((1, 256), np.float32)
    inds[0, 0:64] = 1.0
    inds[0, 192:256] = 1.0
    onesP = np.ones((1, pmax), np.float32)
    zeros64 = np.zeros((64, 384 * 8), np.float16)
    w2t = np.ascontiguousarray(
        w2.reshape(4, 128, 12, 1024).transpose(0, 2, 1, 3)).astype(
            np.float16)
    pidx = np.arange(128, dtype=np.float32) % 64

    in_maps = []
    for k in range(CORES):
        b = k // 2
        sel = sels[k]
        P = len(sel)
        zT = np.zeros((D, pmax), np.float32)
        if P:
            zT[:, :P] = z_what[b, sel].T
        dall = np.where(pres[b], depth[b], EMPTY_DEPTH).astype(
            np.float32).reshape(1, N)
        dsel = np.full((1, pmax), EMPTY_DEPTH, np.float32)
        dsel[0, :P] = depth[b, sel]

        afA = np.zeros((128, 2 * pairs), np.float32)
        afB = np.zeros((128, 2 * pairs), np.float32)
        for p in range(pairs):
            for hf in range(2):
                i = 2 * p + hf
                rows = slice(hf * 64, (hf + 1) * 64)
                if i < P:
                    xc, yc, ww, hh = z_where[b, sel[i]]
                    sx = max(ww, 1e-3)
                    sy = max(hh, 1e-3)
                    tx = 2.0 * xc - 1.0
                    ty = 2.0 * yc - 1.0
                    aY = 63.0 / (415.0 * sy)
                    bY = 31.5 * ((-1.0 - ty) / sy + 1.0)
                    aX = 63.0 / (415.0 * sx)
                    bX = 31.5 * ((-1.0 - tx) / sx + 1.0)
                else:
                    aY = aX = 0.0
                    bY = bX = -5.0
                afA[rows, 2 * p] = aY
                afA[rows, 2 * p + 1] = bY - pidx[rows]
                afB[rows, 2 * p] = aX
                afB[rows, 2 * p + 1] = bX - pidx[rows]

        in_maps.append({
            "zT": zT, "dall": dall, "dsel": dsel, "afA": afA, "afB": afB,
            "w1": w1, "b1c": b1c, "w2t": w2t, "b2r": b2r,
            "iota416": iota416, "inds": inds, "onesP": onesP,
            "zeros": zeros64,
        })

    hasb2 = bool(np.any(b2))
    key = (pmax, hasb2)
    if key not in _CACHE:
        _CACHE[key] = _build(pmax, hasb2)
    nc = _CACHE[key]

    trace = os.environ.get("BASS_KERNEL_TRACE", "0") == "1"
    res = run_bass_kernel_spmd(nc, in_maps, list(range(CORES)), trace=trace)
    if trace and res.exec_time_ns is not None:
        print(f"HW exec time: {res.exec_time_ns} ns")

    recon = np.zeros((B, 3, IMG, IMG), np.float32)
    for b in range(B):
        ct = res.results[2 * b]["canvasT"] + res.results[2 * b + 1]["canvasT"]
        recon[b] = ct.transpose(0, 2, 1)
    return recon


# revision 11
# speedup vs baseline: 1.3722x; 1.0757x over previous
"""Trainium2 Bass kernel for nn_Decoder (AIR-style decoder: per-object MLP
decode -> spatial transformer paste -> depth-softmax compositing).

Sharding: 8 cores = (batch b = core//2) x (half of the 32 object slots).
Host gathers the *present* objects of each half (padded to a common even
count Pmax), ships scalar affine-marshalled grid params, and sums the two
half-canvases per batch at the end. All O(big) math runs on device:
  - what-decoder MLP (bf16 matmuls for the big w2 layer, fp32 accum)
  - sigmoid / exp / softmax normalization
  - separable bilinear STN as two matmul hops (fp32r):
      W[x,h]       = sum_y P[y,(c,x)] * AT[y,h]
      canvasT[w,h] = sum_{n,x} B[x,w] * W[x,h]
  - interpolation grids AT/B built on device: one DVE affine op over an
    iota table + ACT Abs + ACT Relu (hat = relu(1-|py - idx|)), with the
    per-object softmax weight folded into B via ACT scale/bias columns.
The channel loop interleaves the w2 stream with the STN of the previous
channel so DMA and PE overlap. Output per core: transposed partial canvas
[3, 416(w), 416(h)]; host sums core pairs and un-transposes.
"""

import os
import numpy as np

import concourse.bacc as bacc
import concourse.tile as tile
import concourse.mybir as mybir
from concourse.bass_utils import run_bass_kernel_spmd

dt = mybir.dt
F32 = dt.float32
F32R = dt.float32r
BF16 = dt.float16  # fp16: same PE rate as bf16, 8 more mantissa bits
AF = mybir.ActivationFunctionType
ALU = mybir.AluOpType

B, N, D, H1 = 4, 32, 64, 512
S = 64           # decoded patch size
IMG = 416        # canvas size
JD = 3 * S * S   # 12288 decoder output dim
CORES = 8
HALF = N // 2    # 16 object slots per core
EMPTY_DEPTH = -1000.0

_CACHE = {}


def _build(pmax, hasb2):
    pairs = pmax // 2
    nc = bacc.Bacc("TRN2", target_bir_lowering=False, debug=False,
                   num_devices=CORES)

    # ---- per-core inputs ----
    zT_d = nc.declare_dram_parameter("zT", [D, pmax], F32R, isOutput=False)
    dall_d = nc.declare_dram_parameter("dall", [1, N], F32, isOutput=False)
    dsel_d = nc.declare_dram_parameter("dsel", [1, pmax], F32, isOutput=False)
    afA_d = nc.declare_dram_parameter("afA", [128, 2 * pairs], F32, isOutput=False)
    afB_d = nc.declare_dram_parameter("afB", [128, 2 * pairs], F32, isOutput=False)
    # ---- shared (same content on every core) ----
    w1_d = nc.declare_dram_parameter("w1", [D, H1], F32R, isOutput=False)
    b1c_d = nc.declare_dram_parameter("b1c", [128, 4], F32, isOutput=False)
    w2_d = nc.declare_dram_parameter("w2t", [4, 12, 128, 1024], BF16,
                                     isOutput=False)
    b2r_d = nc.declare_dram_parameter("b2r", [1, JD], F32R, isOutput=False)
    iota_d = nc.declare_dram_parameter("iota416", [128, IMG], F32, isOutput=False)
    inds_d = nc.declare_dram_parameter("inds", [1, 256], F32R, isOutput=False)
    onesP_d = nc.declare_dram_parameter("onesP", [1, pmax], F32R, isOutput=False)
    zer_d = nc.declare_dram_parameter("zeros", [128, 384 * 8], BF16,
                                      isOutput=False)
    # ---- output: transposed partial canvas ----
    out_d = nc.declare_dram_parameter("canvasT", [3, IMG, IMG], F32, isOutput=True)

    WCH = [(0, 128), (128, 128), (256, 128), (384, 32)]  # (w0, wsize) chunks

    with tile.TileContext(nc) as tc:
        with tc.tile_pool(name="const", bufs=1) as cp, \
             tc.tile_pool(name="w2s", bufs=3) as w2p, \
             tc.tile_pool(name="work", bufs=1) as wk, \
             tc.tile_pool(name="wpool", bufs=2) as wpl, \
             tc.tile_pool(name="evac", bufs=2) as ev, \
             tc.tile_pool(name="psA", bufs=3, space="PSUM") as psA, \
             tc.tile_pool(name="psW", bufs=2, space="PSUM") as psW, \
             tc.tile_pool(name="psC", bufs=2, space="PSUM") as psC:

            # ---------- load constants / inputs ----------
            zT_t = cp.tile([D, pmax], F32R, tag="zT")
            nc.gpsimd.dma_start(zT_t[:], zT_d[:])
            w1_t = cp.tile([D, H1], F32R, tag="w1")
            nc.gpsimd.dma_start(w1_t[:], w1_d[:])
            b1c_t = cp.tile([128, 4], F32, tag="b1c")
            nc.gpsimd.dma_start(b1c_t[:], b1c_d[:])
            iota_t = cp.tile([128, IMG], F32, tag="iota")
            nc.gpsimd.dma_start(iota_t[:], iota_d[:])
            inds_t = cp.tile([1, 256], F32R, tag="inds")
            nc.gpsimd.dma_start(inds_t[:], inds_d[:])
            onesP_t = cp.tile([1, pmax], F32R, tag="onesP")
            nc.gpsimd.dma_start(onesP_t[:], onesP_d[:])
            dall_t = cp.tile([1, N], F32, tag="dall")
            nc.gpsimd.dma_start(dall_t[:], dall_d[:])
            dsel_t = cp.tile([1, pmax], F32, tag="dsel")
            nc.gpsimd.dma_start(dsel_t[:], dsel_d[:])
            afA_t = cp.tile([128, 2 * pairs], F32, tag="afA")
            nc.gpsimd.dma_start(afA_t[:], afA_d[:])
            afB_t = cp.tile([128, 2 * pairs], F32, tag="afB")
            nc.gpsimd.dma_start(afB_t[:], afB_d[:])

            # ---------- MLP hid: hidT = relu(w1.T @ z + b1), [H1, pmax] ----
            hidT_t = cp.tile([128, 4 * pmax], BF16, tag="hidT")
            for kc in range(4):
                hp = psA.tile([128, pmax], F32, tag="psA")
                nc.tensor.matmul(hp[:], w1_t[:, kc * 128:(kc + 1) * 128],
                                 zT_t[:], start=True, stop=True)
                nc.scalar.activation(hidT_t[:, kc * pmax:(kc + 1) * pmax], hp[:],
                                     AF.Relu, bias=b1c_t[:, kc:kc + 1])

            # ---------- softmax weights s (selected objects) ----------
            eall_t = wk.tile([1, N], F32, tag="eall")
            nc.scalar.activation(eall_t[:], dall_t[:], AF.Exp)
            z_t = wk.tile([1, 1], F32, tag="z")
            nc.vector.tensor_reduce(z_t[:], eall_t[:], mybir.AxisListType.X,
                                    ALU.add)
            zg_t = wk.tile([1, 1], F32, tag="zg")
            nc.vector.tensor_scalar_add(zg_t[:], z_t[:], 1e-30)
            zr_t = wk.tile([1, 1], F32, tag="zr")
            nc.vector.reciprocal(zr_t[:], zg_t[:])
            esel_t = wk.tile([1, pmax], F32, tag="esel")
            nc.scalar.activation(esel_t[:], dsel_t[:], AF.Exp)
            srow_t = wk.tile([1, pmax], F32R, tag="srow")
            nc.vector.tensor_scalar_mul(srow_t[:], esel_t[:], zr_t[:, 0:1])
            # broadcast to per-partition columns: scol[p', pair] (lo=even obj)
            sp = psA.tile([128, pairs], F32, tag="psA")
            nc.tensor.matmul(sp[:], inds_t[0:1, 0:128],
                             srow_t[0:1, 0:pmax:2], start=True, stop=False)
            nc.tensor.matmul(sp[:], inds_t[0:1, 128:256],
                             srow_t[0:1, 1:pmax:2], start=False, stop=True)
            scol_t = wk.tile([128, pairs], F32, tag="scol")
            nc.scalar.copy(scol_t[:], sp[:])
            nscol_t = wk.tile([128, pairs], F32, tag="nscol")
            nc.scalar.mul(nscol_t[:], scol_t[:], -1.0)

            # ---------- grids ----------
            AT_t = wk.tile([128, IMG * pairs], BF16, tag="AT")
            Bg_t = wk.tile([128, IMG * pairs], BF16, tag="Bg")
            for p in range(pairs):
                sl = slice(p * IMG, (p + 1) * IMG)
                mA = ev.tile([128, IMG], F32, tag="m")
                nc.vector.tensor_scalar(mA[:], iota_t[:],
                                        afA_t[:, 2 * p:2 * p + 1],
                                        afA_t[:, 2 * p + 1:2 * p + 2],
                                        ALU.mult, ALU.add)
                tA = ev.tile([128, IMG], F32, tag="t")
                nc.scalar.activation(tA[:], mA[:], AF.Abs)
                nc.scalar.activation(AT_t[:, sl], tA[:], AF.Relu,
                                     bias=1.0, scale=-1.0)
                mB = ev.tile([128, IMG], F32, tag="m")
                nc.vector.tensor_scalar(mB[:], iota_t[:],
                                        afB_t[:, 2 * p:2 * p + 1],
                                        afB_t[:, 2 * p + 1:2 * p + 2],
                                        ALU.mult, ALU.add)
                tB = ev.tile([128, IMG], F32, tag="t")
                nc.scalar.activation(tB[:], mB[:], AF.Abs)
                nc.scalar.activation(Bg_t[:, sl], tB[:], AF.Relu,
                                     bias=scol_t[:, p:p + 1],
                                     scale=nscol_t[:, p:p + 1])

            # ---------- per-channel: dec -> patches -> step1 -> step2 ------
            # Pp [128(y: even obj 0:64, odd 64:128), 384*pairs]: cols
            # c*128+[0:64] = even object's x, +[64:128] = odd's x; the other
            # half of each 128-col block stays zero, so ONE K=128 matmul per
            # (pair, c) contracts both objects' y and writes disjoint x rows.
            Pp_t = wk.tile([128, 384 * pairs], BF16, tag="Pp")
            nc.gpsimd.dma_start(Pp_t[:], zer_d[:, 0:384 * pairs])

            for c in range(3):
                # --- dec_c = sigmoid(hid @ w2[:, c-block] + b2) ---
                dec_c = w2p.tile([pmax, 4096], BF16, tag="dec_c")
                for gg in range(4):
                    g = c * 4 + gg
                    w2g = []
                    for kt in range(4):
                        t = w2p.tile([128, 1024], BF16, tag=f"w2_{kt}")
                        nc.sync.dma_start(t[:], w2_d[kt, g])
                        w2g.append(t)
                    if hasb2:
                        b2g = w2p.tile([1, 1024], F32R, tag="b2g")
                        nc.sync.dma_start(b2g[:],
                                          b2r_d[0:1, g * 1024:(g + 1) * 1024])
                    for jl in range(2):
                        dp = psA.tile([pmax, 512], F32, tag="psA")
                        for kt in range(4):
                            nc.tensor.matmul(
                                dp[:], hidT_t[:, kt * pmax:(kt + 1) * pmax],
                                w2g[kt][:, jl * 512:(jl + 1) * 512],
                                start=(kt == 0), stop=(kt == 3 and not hasb2))
                        if hasb2:
                            nc.tensor.matmul(dp[:], onesP_t[:],
                                             b2g[:, jl * 512:(jl + 1) * 512],
                                             start=False, stop=True)
                        nc.scalar.activation(
                            dec_c[:, (gg * 2 + jl) * 512:
                                  (gg * 2 + jl + 1) * 512],
                            dp[:], AF.Sigmoid)
                # --- patch gather for this channel ---
                for p in range(pairs):
                    base = p * 384
                    for hf in range(2):
                        n = 2 * p + hf
                        src = dec_c[n:n + 1, :].rearrange(
                            "o (y x) -> o y x", y=S, x=S)
                        dst = Pp_t[hf * 64:(hf + 1) * 64,
                                   base + c * 128 + hf * 64:
                                   base + c * 128 + (hf + 1) * 64]
                        nc.gpsimd.dma_start(dst.opt(), src.opt())
                # --- step 1: W[pair] [128(x-halves), IMG(h)] ---
                Wt = wpl.tile([128, IMG * pairs], BF16, tag="W")
                for p in range(pairs):
                    base = p * 384
                    psl = slice(p * IMG, (p + 1) * IMG)
                    wp = psW.tile([128, IMG], F32, tag="psW")
                    nc.tensor.matmul(
                        wp[:], Pp_t[:, base + c * 128:base + (c + 1) * 128],
                        AT_t[:, psl], start=True, stop=True)
                    nc.vector.tensor_copy(Wt[:, psl], wp[:])
                # --- step 2: canvasT[c][w, h] ---
                for (w0, wsz) in WCH:
                    cv = psC.tile([wsz, IMG], F32, tag="psC")
                    for p in range(pairs):
                        nc.tensor.matmul(
                            cv[:], Bg_t[:, p * IMG + w0:p * IMG + w0 + wsz],
                            Wt[:, p * IMG:(p + 1) * IMG],
                            start=(p == 0), stop=(p == pairs - 1))
                    ot = ev.tile([wsz, IMG], F32, tag="cv")
                    nc.vector.tensor_copy(ot[:], cv[:])
                    nc.sync.dma_start(out_d[c, w0:w0 + wsz, :], ot[:])

    nc.compile()
    return nc


def kernel(z_where, z_present, z_what, z_depth, w1, b1, w2, b2):
    z_where = np.asarray(z_where, dtype=np.float32)
    z_present = np.asarray(z_present)
    z_what = np.asarray(z_what, dtype=np.float32)
    z_depth = np.asarray(z_depth, dtype=np.float32)
    w1 = np.ascontiguousarray(np.asarray(w1, dtype=np.float32))
    b1 = np.asarray(b1, dtype=np.float32)
    w2 = np.ascontiguousarray(np.asarray(w2, dtype=np.float32))
    b2 = np.asarray(b2, dtype=np.float32)

    pres = z_present.reshape(B, N) > 0
    depth = z_depth.reshape(B, N)

    # per-core object selection (present only, sorted by y-center)
    sels = []
    for k in range(CORES):
        b, half = k // 2, k % 2
        cand = [n for n in range(half * HALF, (half + 1) * HALF) if pres[b, n]]
        cand.sort(key=lambda n: z_where[b, n, 1])
        sels.append(cand)
    pmax = max(2, max((len(s) for s in sels), default=0))
    if pmax % 2:
        pmax += 1
    pairs = pmax // 2

    # shared constants
    b1c = np.ascontiguousarray(b1.reshape(4, 128).T)
    b2r = b2.reshape(1, JD)
    iota416 = np.ascontiguousarray(
        np.broadcast_to(np.arange(IMG, dtype=np.float32), (128, IMG)))
    inds = np.zeros((1, 256), np.float32)
    inds[0, 0:64] = 1.0
    inds[0, 192:256] = 1.0
    onesP = np.ones((1, pmax), np.float32)
    zeros64 = np.zeros((128, 384 * 8), np.float16)
    w2t = np.ascontiguousarray(
        w2.reshape(4, 128, 12, 1024).transpose(0, 2, 1, 3)).astype(
            np.float16)
    pidx = np.arange(128, dtype=np.float32) % 64

    in_maps = []
    for k in range(CORES):
        b = k // 2
        sel = sels[k]
        P = len(sel)
        zT = np.zeros((D, pmax), np.float32)
        if P:
            zT[:, :P] = z_what[b, sel].T
        dall = np.where(pres[b], depth[b], EMPTY_DEPTH).astype(
            np.float32).reshape(1, N)
        dsel = np.full((1, pmax), EMPTY_DEPTH, np.float32)
        dsel[0, :P] = depth[b, sel]

        afA = np.zeros((128, 2 * pairs), np.float32)
        afB = np.zeros((128, 2 * pairs), np.float32)
        for p in range(pairs):
            for hf in range(2):
                i = 2 * p + hf
                rows = slice(hf * 64, (hf + 1) * 64)
                if i < P:
                    xc, yc, ww, hh = z_where[b, sel[i]]
                    sx = max(ww, 1e-3)
                    sy = max(hh, 1e-3)
                    tx = 2.0 * xc - 1.0
                    ty = 2.0 * yc - 1.0
                    aY = 63.0 / (415.0 * sy)
                    bY = 31.5 * ((-1.0 - ty) / sy + 1.0)
                    aX = 63.0 / (415.0 * sx)
                    bX = 31.5 * ((-1.0 - tx) / sx + 1.0)
                else:
                    aY = aX = 0.0
                    bY = bX = -5.0
                afA[rows, 2 * p] = aY
                afA[rows, 2 * p + 1] = bY - pidx[rows]
                afB[rows, 2 * p] = aX
                afB[rows, 2 * p + 1] = bX - pidx[rows]

        in_maps.append({
            "zT": zT, "dall": dall, "dsel": dsel, "afA": afA, "afB": afB,
            "w1": w1, "b1c": b1c, "w2t": w2t, "b2r": b2r,
            "iota416": iota416, "inds": inds, "onesP": onesP,
            "zeros": zeros64,
        })

    hasb2 = bool(np.any(b2))
    key = (pmax, hasb2)
    if key not in _CACHE:
        _CACHE[key] = _build(pmax, hasb2)
    nc = _CACHE[key]

    trace = os.environ.get("BASS_KERNEL_TRACE", "0") == "1"
    res = run_bass_kernel_spmd(nc, in_maps, list(range(CORES)), trace=trace)
    if trace and res.exec_time_ns is not None:
        print(f"HW exec time: {res.exec_time_ns} ns")

    recon = np.zeros((B, 3, IMG, IMG), np.float32)
    for b in range(B):
        ct = res.results[2 * b]["canvasT"] + res.results[2 * b + 1]["canvasT"]
        recon[b] = ct.transpose(0, 2, 1)
    return recon


# revision 13
# speedup vs baseline: 1.4380x; 1.0479x over previous
"""Trainium2 Bass kernel for nn_Decoder (AIR-style decoder: per-object MLP
decode -> spatial transformer paste -> depth-softmax compositing).

Sharding: 8 cores = (batch b = core//2) x (half of the 32 object slots).
Host gathers the *present* objects of each half (padded to a common even
count Pmax), ships scalar affine-marshalled grid params, and sums the two
half-canvases per batch at the end. All O(big) math runs on device:
  - what-decoder MLP (bf16 matmuls for the big w2 layer, fp32 accum)
  - sigmoid / exp / softmax normalization
  - separable bilinear STN as two matmul hops (fp32r):
      W[x,h]       = sum_y P[y,(c,x)] * AT[y,h]
      canvasT[w,h] = sum_{n,x} B[x,w] * W[x,h]
  - interpolation grids AT/B built on device: one DVE affine op over an
    iota table + ACT Abs + ACT Relu (hat = relu(1-|py - idx|)), with the
    per-object softmax weight folded into B via ACT scale/bias columns.
The channel loop interleaves the w2 stream with the STN of the previous
channel so DMA and PE overlap. Output per core: transposed partial canvas
[3, 416(w), 416(h)]; host sums core pairs and un-transposes.
"""

import os
import numpy as np

import concourse.bacc as bacc
import concourse.tile as tile
import concourse.mybir as mybir
from concourse.bass_utils import run_bass_kernel_spmd

dt = mybir.dt
F32 = dt.float32
F32R = dt.float32r
BF16 = dt.float16  # fp16: same PE rate as bf16, 8 more mantissa bits
AF = mybir.ActivationFunctionType
ALU = mybir.AluOpType

B, N, D, H1 = 4, 32, 64, 512
S = 64           # decoded patch size
IMG = 416        # canvas size
JD = 3 * S * S   # 12288 decoder output dim
CORES = 8
HALF = N // 2    # 16 object slots per core
EMPTY_DEPTH = -1000.0

_CACHE = {}


def _build(pmax, hasb2):
    pairs = pmax // 2
    nc = bacc.Bacc("TRN2", target_bir_lowering=False, debug=False,
                   num_devices=CORES)

    # ---- per-core inputs ----
    zT_d = nc.declare_dram_parameter("zT", [D, pmax], F32R, isOutput=False)
    dall_d = nc.declare_dram_parameter("dall", [1, N], F32, isOutput=False)
    dsel_d = nc.declare_dram_parameter("dsel", [1, pmax], F32, isOutput=False)
    afA_d = nc.declare_dram_parameter("afA", [128, 2 * pairs], F32, isOutput=False)
    afB_d = nc.declare_dram_parameter("afB", [128, 2 * pairs], F32, isOutput=False)
    # ---- shared (same content on every core) ----
    w1_d = nc.declare_dram_parameter("w1", [D, H1], F32R, isOutput=False)
    b1c_d = nc.declare_dram_parameter("b1c", [128, 4], F32, isOutput=False)
    w2_d = nc.declare_dram_parameter("w2t", [4, 6, 128, 2048], BF16,
                                     isOutput=False)
    b2r_d = nc.declare_dram_parameter("b2r", [1, JD], F32R, isOutput=False)
    iota_d = nc.declare_dram_parameter("iota416", [128, IMG], F32, isOutput=False)
    inds_d = nc.declare_dram_parameter("inds", [1, 256], F32R, isOutput=False)
    onesP_d = nc.declare_dram_parameter("onesP", [1, pmax], F32R, isOutput=False)
    zer_d = nc.declare_dram_parameter("zeros", [128, 384 * 8], BF16,
                                      isOutput=False)
    # ---- output: transposed partial canvas ----
    out_d = nc.declare_dram_parameter("canvasT", [3, IMG, IMG], F32, isOutput=True)

    WCH = [(0, 128), (128, 128), (256, 128), (384, 32)]  # (w0, wsize) chunks

    with tile.TileContext(nc) as tc:
        with tc.tile_pool(name="const", bufs=1) as cp, \
             tc.tile_pool(name="w2s", bufs=3) as w2p, \
             tc.tile_pool(name="work", bufs=1) as wk, \
             tc.tile_pool(name="wpool", bufs=2) as wpl, \
             tc.tile_pool(name="evac", bufs=2) as ev, \
             tc.tile_pool(name="psA", bufs=3, space="PSUM") as psA, \
             tc.tile_pool(name="psW", bufs=2, space="PSUM") as psW, \
             tc.tile_pool(name="psC", bufs=3, space="PSUM") as psC:

            # ---------- load constants / inputs ----------
            zT_t = cp.tile([D, pmax], F32R, tag="zT")
            nc.gpsimd.dma_start(zT_t[:], zT_d[:])
            w1_t = cp.tile([D, H1], F32R, tag="w1")
            nc.gpsimd.dma_start(w1_t[:], w1_d[:])
            b1c_t = cp.tile([128, 4], F32, tag="b1c")
            nc.gpsimd.dma_start(b1c_t[:], b1c_d[:])
            iota_t = cp.tile([128, IMG], F32, tag="iota")
            nc.gpsimd.dma_start(iota_t[:], iota_d[:])
            inds_t = cp.tile([1, 256], F32R, tag="inds")
            nc.gpsimd.dma_start(inds_t[:], inds_d[:])
            onesP_t = cp.tile([1, pmax], F32R, tag="onesP")
            nc.gpsimd.dma_start(onesP_t[:], onesP_d[:])
            dall_t = cp.tile([1, N], F32, tag="dall")
            nc.gpsimd.dma_start(dall_t[:], dall_d[:])
            dsel_t = cp.tile([1, pmax], F32, tag="dsel")
            nc.gpsimd.dma_start(dsel_t[:], dsel_d[:])
            afA_t = cp.tile([128, 2 * pairs], F32, tag="afA")
            nc.gpsimd.dma_start(afA_t[:], afA_d[:])
            afB_t = cp.tile([128, 2 * pairs], F32, tag="afB")
            nc.gpsimd.dma_start(afB_t[:], afB_d[:])

            # ---------- MLP hid: hidT = relu(w1.T @ z + b1), [H1, pmax] ----
            hidT_t = cp.tile([128, 4 * pmax], BF16, tag="hidT")
            for kc in range(4):
                hp = psA.tile([128, pmax], F32, tag="psA")
                nc.tensor.matmul(hp[:], w1_t[:, kc * 128:(kc + 1) * 128],
                                 zT_t[:], start=True, stop=True)
                nc.scalar.activation(hidT_t[:, kc * pmax:(kc + 1) * pmax], hp[:],
                                     AF.Relu, bias=b1c_t[:, kc:kc + 1])

            # ---------- softmax weights s (selected objects) ----------
            eall_t = wk.tile([1, N], F32, tag="eall")
            nc.scalar.activation(eall_t[:], dall_t[:], AF.Exp)
            z_t = wk.tile([1, 1], F32, tag="z")
            nc.vector.tensor_reduce(z_t[:], eall_t[:], mybir.AxisListType.X,
                                    ALU.add)
            zg_t = wk.tile([1, 1], F32, tag="zg")
            nc.vector.tensor_scalar_add(zg_t[:], z_t[:], 1e-30)
            zr_t = wk.tile([1, 1], F32, tag="zr")
            nc.vector.reciprocal(zr_t[:], zg_t[:])
            esel_t = wk.tile([1, pmax], F32, tag="esel")
            nc.scalar.activation(esel_t[:], dsel_t[:], AF.Exp)
            srow_t = wk.tile([1, pmax], F32R, tag="srow")
            nc.vector.tensor_scalar_mul(srow_t[:], esel_t[:], zr_t[:, 0:1])
            # broadcast to per-partition columns: scol[p', pair] (lo=even obj)
            sp = psA.tile([128, pairs], F32, tag="psA")
            nc.tensor.matmul(sp[:], inds_t[0:1, 0:128],
                             srow_t[0:1, 0:pmax:2], start=True, stop=False)
            nc.tensor.matmul(sp[:], inds_t[0:1, 128:256],
                             srow_t[0:1, 1:pmax:2], start=False, stop=True)
            scol_t = wk.tile([128, pairs], F32, tag="scol")
            nc.scalar.copy(scol_t[:], sp[:])
            nscol_t = wk.tile([128, pairs], F32, tag="nscol")
            nc.scalar.mul(nscol_t[:], scol_t[:], -1.0)

            # ---------- grids ----------
            AT_t = wk.tile([128, IMG * pairs], BF16, tag="AT")
            Bg_t = wk.tile([128, IMG * pairs], BF16, tag="Bg")
            tAll_A = wk.tile([128, IMG * pairs], F32, tag="tA")
            tAll_B = wk.tile([128, IMG * pairs], F32, tag="tB")
            for ax, (af_t, tAll) in enumerate(((afA_t, tAll_A),
                                               (afB_t, tAll_B))):
                for p in range(pairs):
                    sl = slice(p * IMG, (p + 1) * IMG)
                    m = ev.tile([128, IMG], F32, tag="m")
                    nc.vector.tensor_scalar(m[:], iota_t[:],
                                            af_t[:, 2 * p:2 * p + 1],
                                            af_t[:, 2 * p + 1:2 * p + 2],
                                            ALU.mult, ALU.add)
                    nc.scalar.activation(tAll[:, sl], m[:], AF.Abs)
            nc.scalar.activation(AT_t[:], tAll_A[:], AF.Relu,
                                 bias=1.0, scale=-1.0)
            for p in range(pairs):
                sl = slice(p * IMG, (p + 1) * IMG)
                nc.scalar.activation(Bg_t[:, sl], tAll_B[:, sl], AF.Relu,
                                     bias=scol_t[:, p:p + 1],
                                     scale=nscol_t[:, p:p + 1])

            # ---------- per-channel: dec -> patches -> step1 -> step2 ------
            # Pp [128(y: even obj 0:64, odd 64:128), 384*pairs]: cols
            # c*128+[0:64] = even object's x, +[64:128] = odd's x; the other
            # half of each 128-col block stays zero, so ONE K=128 matmul per
            # (pair, c) contracts both objects' y and writes disjoint x rows.
            Pp_t = wk.tile([128, 384 * pairs], BF16, tag="Pp")
            nc.gpsimd.dma_start(Pp_t[:], zer_d[:, 0:384 * pairs])

            for c in range(3):
                # --- dec_c = sigmoid(hid @ w2[:, c-block] + b2) ---
                dec_c = w2p.tile([pmax, 4096], BF16, tag="dec_c")
                for gg in range(2):
                    g2 = c * 2 + gg
                    w2g = []
                    for kt in range(4):
                        t = w2p.tile([128, 2048], BF16, tag=f"w2_{kt}")
                        nc.sync.dma_start(t[:], w2_d[kt, g2])
                        w2g.append(t)
                    if hasb2:
                        b2g = w2p.tile([1, 2048], F32R, tag="b2g")
                        nc.sync.dma_start(
                            b2g[:], b2r_d[0:1, g2 * 2048:(g2 + 1) * 2048])
                    for jl in range(4):
                        dp = psA.tile([pmax, 512], F32, tag="psA")
                        for kt in range(4):
                            nc.tensor.matmul(
                                dp[:], hidT_t[:, kt * pmax:(kt + 1) * pmax],
                                w2g[kt][:, jl * 512:(jl + 1) * 512],
                                start=(kt == 0), stop=(kt == 3 and not hasb2))
                        if hasb2:
                            nc.tensor.matmul(dp[:], onesP_t[:],
                                             b2g[:, jl * 512:(jl + 1) * 512],
                                             start=False, stop=True)
                        nc.scalar.activation(
                            dec_c[:, (gg * 4 + jl) * 512:
                                  (gg * 4 + jl + 1) * 512],
                            dp[:], AF.Sigmoid)
                # --- patch gather for this channel ---
                for p in range(pairs):
                    base = p * 384
                    for hf in range(2):
                        n = 2 * p + hf
                        src = dec_c[n:n + 1, :].rearrange(
                            "o (y x) -> o y x", y=S, x=S)
                        dst = Pp_t[hf * 64:(hf + 1) * 64,
                                   base + c * 128 + hf * 64:
                                   base + c * 128 + (hf + 1) * 64]
                        nc.gpsimd.dma_start(dst.opt(), src.opt())
                # --- step 1: W[pair] [128(x-halves), IMG(h)] ---
                Wt = wpl.tile([128, IMG * pairs], BF16, tag="W")
                for p in range(pairs):
                    base = p * 384
                    psl = slice(p * IMG, (p + 1) * IMG)
                    wp = psW.tile([128, IMG], F32, tag="psW")
                    nc.tensor.matmul(
                        wp[:], Pp_t[:, base + c * 128:base + (c + 1) * 128],
                        AT_t[:, psl], start=True, stop=True)
                    nc.vector.tensor_copy(Wt[:, psl], wp[:])
                # --- step 2: canvasT[c][w, h] ---
                for (w0, wsz) in WCH:
                    cv = psC.tile([wsz, IMG], F32, tag="psC")
                    for p in range(pairs):
                        nc.tensor.matmul(
                            cv[:], Bg_t[:, p * IMG + w0:p * IMG + w0 + wsz],
                            Wt[:, p * IMG:(p + 1) * IMG],
                            start=(p == 0), stop=(p == pairs - 1))
                    ot = ev.tile([wsz, IMG], F32, tag="cv")
                    nc.vector.tensor_copy(ot[:], cv[:])
                    nc.sync.dma_start(out_d[c, w0:w0 + wsz, :], ot[:])

    nc.compile()
    return nc


def kernel(z_where, z_present, z_what, z_depth, w1, b1, w2, b2):
    z_where = np.asarray(z_where, dtype=np.float32)
    z_present = np.asarray(z_present)
    z_what = np.asarray(z_what, dtype=np.float32)
    z_depth = np.asarray(z_depth, dtype=np.float32)
    w1 = np.ascontiguousarray(np.asarray(w1, dtype=np.float32))
    b1 = np.asarray(b1, dtype=np.float32)
    w2 = np.ascontiguousarray(np.asarray(w2, dtype=np.float32))
    b2 = np.asarray(b2, dtype=np.float32)

    pres = z_present.reshape(B, N) > 0
    depth = z_depth.reshape(B, N)

    # per-core object selection (present only, sorted by y-center)
    sels = []
    for k in range(CORES):
        b, half = k // 2, k % 2
        cand = [n for n in range(half * HALF, (half + 1) * HALF) if pres[b, n]]
        cand.sort(key=lambda n: z_where[b, n, 1])
        sels.append(cand)
    pmax = max(2, max((len(s) for s in sels), default=0))
    if pmax % 2:
        pmax += 1
    pairs = pmax // 2

    # shared constants
    b1c = np.ascontiguousarray(b1.reshape(4, 128).T)
    b2r = b2.reshape(1, JD)
    iota416 = np.ascontiguousarray(
        np.broadcast_to(np.arange(IMG, dtype=np.float32), (128, IMG)))
    inds = np.zeros((1, 256), np.float32)
    inds[0, 0:64] = 1.0
    inds[0, 192:256] = 1.0
    onesP = np.ones((1, pmax), np.float32)
    zeros64 = np.zeros((128, 384 * 8), np.float16)
    w2t = np.ascontiguousarray(
        w2.reshape(4, 128, 6, 2048).transpose(0, 2, 1, 3)).astype(
            np.float16)
    pidx = np.arange(128, dtype=np.float32) % 64

    in_maps = []
    for k in range(CORES):
        b = k // 2
        sel = sels[k]
        P = len(sel)
        zT = np.zeros((D, pmax), np.float32)
        if P:
            zT[:, :P] = z_what[b, sel].T
        dall = np.where(pres[b], depth[b], EMPTY_DEPTH).astype(
            np.float32).reshape(1, N)
        dsel = np.full((1, pmax), EMPTY_DEPTH, np.float32)
        dsel[0, :P] = depth[b, sel]

        afA = np.zeros((128, 2 * pairs), np.float32)
        afB = np.zeros((128, 2 * pairs), np.float32)
        for p in range(pairs):
            for hf in range(2):
                i = 2 * p + hf
                rows = slice(hf * 64, (hf + 1) * 64)
                if i < P:
                    xc, yc, ww, hh = z_where[b, sel[i]]
                    sx = max(ww, 1e-3)
                    sy = max(hh, 1e-3)
                    tx = 2.0 * xc - 1.0
                    ty = 2.0 * yc - 1.0
                    aY = 63.0 / (415.0 * sy)
                    bY = 31.5 * ((-1.0 - ty) / sy + 1.0)
                    aX = 63.0 / (415.0 * sx)
                    bX = 31.5 * ((-1.0 - tx) / sx + 1.0)
                else:
                    aY = aX = 0.0
                    bY = bX = -5.0
                afA[rows, 2 * p] = aY
                afA[rows, 2 * p + 1] = bY - pidx[rows]
                afB[rows, 2 * p] = aX
                afB[rows, 2 * p + 1] = bX - pidx[rows]

        in_maps.append({
            "zT": zT, "dall": dall, "dsel": dsel, "afA": afA, "afB": afB,
            "w1": w1, "b1c": b1c, "w2t": w2t, "b2r": b2r,
            "iota416": iota416, "inds": inds, "onesP": onesP,
            "zeros": zeros64,
        })

    hasb2 = bool(np.any(b2))
    key = (pmax, hasb2)
    if key not in _CACHE:
        _CACHE[key] = _build(pmax, hasb2)
    nc = _CACHE[key]

    trace = os.environ.get("BASS_KERNEL_TRACE", "0") == "1"
    res = run_bass_kernel_spmd(nc, in_maps, list(range(CORES)), trace=trace)
    if trace and res.exec_time_ns is not None:
        print(f"HW exec time: {res.exec_time_ns} ns")

    recon = np.zeros((B, 3, IMG, IMG), np.float32)
    for b in range(B):
        ct = res.results[2 * b]["canvasT"] + res.results[2 * b + 1]["canvasT"]
        recon[b] = ct.transpose(0, 2, 1)
    return recon
